# revision 8
# baseline (speedup 1.0000x reference)
"""Multi-head attention (B=8, S=2048, D=512, H=8, DH=64) on 8 TRN2 NeuronCores.

Data-parallel over batch; per core everything is transposed (feature on
partitions) so softmax reductions ride the TensorE contraction axis.

v2 schedule (trace-driven rework of the staged baseline):
  - Steady state is ACT-bound at ~1005ns per [128,1024] exp; PE runs
    ~830-940ns/half-jc, so filler projection work must be fine-grained.
  - PV trails the exp stream by TWO j-chunks (was 1), so PV matmuls never
    wait on the just-issued exp (-120ns/jc of PE stall).
  - Lead-in only computes what head 0 jc0 needs: Q pair-0 (full), K pair-0
    column block 0, V chunks 0-3; wq/wk pair-0 column slices are loaded as
    separate tiles so the first matmuls don't wait on the full weight DMA.
    K blocks 1-3, V 4-15 become early head-0 fillers with per-jc deadlines.
  - Output projection is split by contraction block: kt0+kt1 accumulate
    into PSUM during heads 5-6 and stage to SBUF (bf16), kt2 adds in during
    head 7, so the tail only runs kt3 matmuls + one fused
    (po + bias) + stage combine per output chunk (DVE/GpSimd split).
  - Head-7 normalization avoids the slow [1,2048] single-partition ops:
    DMA the PSUM denominator row into a [128,16] tile, reciprocal there,
    DRAM partition-broadcast back, multiply out of PSUM.
"""

import numpy as np
import ml_dtypes

B, S, D = 8, 2048, 512
H, DH = 8, 64
INNER = H * DH
SCALE = DH ** -0.5

N_CORES = 8
NDT = D // 128   # 4 contraction tiles
NSC = S // 128   # 16 s-chunks (j-chunks)
NST = S // 512   # 4 s-tiles


def _build_kernel():
    import concourse.bass as bass
    import concourse.mybir as mybir
    import concourse.tile as tile
    from concourse import bacc

    bf16 = mybir.dt.bfloat16
    f32 = mybir.dt.float32
    Exp = mybir.ActivationFunctionType.Exp
    Add = mybir.AluOpType.add

    nc = bacc.Bacc()

    xT = nc.declare_dram_parameter("xT", [D, S], bf16, isOutput=False)
    wq = nc.declare_dram_parameter("wq", [D, INNER], bf16, isOutput=False)
    wk = nc.declare_dram_parameter("wk", [D, INNER], bf16, isOutput=False)
    wv = nc.declare_dram_parameter("wv", [D, INNER], bf16, isOutput=False)
    wo = nc.declare_dram_parameter("wo", [INNER, D], bf16, isOutput=False)
    bo = nc.declare_dram_parameter("bo", [NDT, 128, 1], f32, isOutput=False)
    out = nc.declare_dram_parameter("out", [D, S], f32, isOutput=True)
    den_dram = nc.dram_tensor("den_scratch", [H, S], f32)

    with tile.TileContext(nc) as tc:
        with (
            tc.tile_pool(name="weights", bufs=1) as wpool,
            tc.tile_pool(name="acts", bufs=1) as apool,
            tc.tile_pool(name="et", bufs=4) as epool,
            tc.tile_pool(name="small", bufs=2) as spool,
            tc.tile_pool(name="ostage", bufs=2) as opool,
            tc.tile_pool(name="psA", bufs=2, space="PSUM") as psA,
            tc.tile_pool(name="psV", bufs=1, space="PSUM") as psV,
        ):
            # ---- SBUF tiles ----
            # wq/wk pair-0 column slice gets its own tile so lead-in matmuls
            # depend only on the small early DMA, not the full weight load.
            wqA = [wpool.tile([128, 128], bf16, name=f"wqA{d}", tag=f"wqA{d}")
                   for d in range(NDT)]
            wkA = [wpool.tile([128, 128], bf16, name=f"wkA{d}", tag=f"wkA{d}")
                   for d in range(NDT)]
            wqB = [wpool.tile([128, INNER - 128], bf16, name=f"wqB{d}",
                              tag=f"wqB{d}") for d in range(NDT)]
            wkB = [wpool.tile([128, INNER - 128], bf16, name=f"wkB{d}",
                              tag=f"wkB{d}") for d in range(NDT)]
            xT_s = [[wpool.tile([128, S // 2], bf16, name=f"xT{d}_{hf}",
                                tag=f"xT{d}_{hf}") for hf in range(2)]
                    for d in range(NDT)]
            wv_s = [wpool.tile([128, INNER], bf16, name=f"wv{d}", tag=f"wv{d}")
                    for d in range(NDT)]
            wo_s = [wpool.tile([128, D], bf16, name=f"wo{d}", tag=f"wo{d}")
                    for d in range(NDT)]
            bo_s = [wpool.tile([128, 1], f32, name=f"bo{d}", tag=f"bo{d}")
                    for d in range(NDT)]

            def wsel(w_pair, t):
                """(tile list, col offset) for head-pair t's projection."""
                if t == 0:
                    return (wqA if w_pair == "q" else wkA), 0
                return (wqB if w_pair == "q" else wkB), (t - 1) * 128

            # ---- prioritized input DMA ----
            for d in range(NDT):
                sl = slice(d * 128, (d + 1) * 128)
                nc.sync.dma_start(out=wqA[d][:], in_=wq[sl, 0:128])
                nc.scalar.dma_start(out=wkA[d][:], in_=wk[sl, 0:128])
            for d in range(NDT):
                sl = slice(d * 128, (d + 1) * 128)
                nc.sync.dma_start(out=xT_s[d][0][:], in_=xT[sl, 0:S // 2])
                nc.scalar.dma_start(out=xT_s[d][1][:], in_=xT[sl, S // 2:])
            for d in range(NDT):
                sl = slice(d * 128, (d + 1) * 128)
                nc.gpsimd.dma_start(out=wv_s[d][:], in_=wv[sl, :])
            for d in range(NDT):
                sl = slice(d * 128, (d + 1) * 128)
                nc.sync.dma_start(out=wqB[d][:], in_=wq[sl, 128:])
                nc.scalar.dma_start(out=wkB[d][:], in_=wk[sl, 128:])
            for d in range(NDT):
                sl = slice(d * 128, (d + 1) * 128)
                nc.gpsimd.dma_start(out=wo_s[d][:], in_=wo[sl, :])
                nc.gpsimd.dma_start(out=bo_s[d][:], in_=bo[d, :, :])

            # ---- QKV target tiles ----
            qt_lo = [apool.tile([128, S], bf16, name=f"qlo{t}", tag=f"qlo{t}")
                     for t in range(NDT)]
            kt_lo = [apool.tile([128, S], bf16, name=f"klo{t}", tag=f"klo{t}")
                     for t in range(NDT)]
            qt_hi = [apool.tile([128, S], bf16, name=f"qhi{t}", tag=f"qhi{t}")
                     for t in range(NDT)]
            kt_hi = [apool.tile([128, S], bf16, name=f"khi{t}", tag=f"khi{t}")
                     for t in range(NDT)]
            v_aug = [apool.tile([128, H * (DH + 1)], bf16, name=f"va{m}",
                                tag=f"va{m}") for m in range(NSC)]
            ot = [apool.tile([128, S], bf16, name=f"ot{t}", tag=f"ot{t}")
                  for t in range(NDT)]
            # out-projection kt{0,1}+kt2 partial sums, bf16 staging
            stg = [[wpool.tile([128, 1024], bf16, name=f"stg{ch}_{hf}",
                               tag=f"stg{ch}_{hf}") for hf in range(2)]
                   for ch in range(NDT)]

            # PE warm-up + GpSimd ucode preload (lazy LOAD_LIB is ~7us)
            junk_sb = wpool.tile([128, 512], bf16, name="junk", tag="junk")
            nc.vector.memset(junk_sb[:, :], 0.0)
            pre_bc = wpool.tile([2, 16], f32, name="prebc", tag="prebc")
            nc.vector.memset(pre_bc[:, :], 1.0)
            nc.gpsimd.partition_broadcast(pre_bc[:, :], pre_bc[0:1, :],
                                          channels=2)
            junk_ps = psV.tile([128, 4 * 512], f32, name="junkps", tag="pv")
            for k in range(10):
                nc.tensor.matmul(
                    junk_ps[:, (k % 4) * 512:(k % 4 + 1) * 512],
                    lhsT=junk_sb[:, 0:128],
                    rhs=junk_sb[:, :],
                )

            # ---- projection piece helpers ----
            def qk_half(w_pair, dst, t, half):
                """One s-half (1024 cols) of a Q/K projection: 8 matmuls."""
                w_s, co = wsel(w_pair, t)
                pa = psA.tile([128, 1024], f32, name="pa", tag="pa")
                for nn in range(2):
                    s0 = nn * 512
                    for d in range(NDT):
                        nc.tensor.matmul(
                            pa[:, nn * 512:(nn + 1) * 512],
                            lhsT=w_s[d][:, co:co + 128],
                            rhs=xT_s[d][half][:, s0:s0 + 512],
                            start=(d == 0),
                            stop=(d == NDT - 1),
                        )
                nc.vector.tensor_copy(
                    dst[t][:, half * 1024:(half + 1) * 1024], pa[:, :])

            def qk_piece(w_pair, dst, t, half, nn):
                """4-matmul piece: one 512-col block of a Q/K projection."""
                w_s, co = wsel(w_pair, t)
                pa = psA.tile([128, 1024], f32, name="pa", tag="pa")
                s0 = nn * 512
                for d in range(NDT):
                    nc.tensor.matmul(
                        pa[:, 0:512],
                        lhsT=w_s[d][:, co:co + 128],
                        rhs=xT_s[d][half][:, s0:s0 + 512],
                        start=(d == 0),
                        stop=(d == NDT - 1),
                    )
                off = half * 1024 + nn * 512
                nc.vector.tensor_copy(dst[t][:, off:off + 512], pa[:, 0:512])

            def swap_half(src_lo, src_hi, t, sh):
                """DMA-swap one 1024-wide s-half of the lo replica into hi."""
                s0, s1 = sh * 1024, (sh + 1) * 1024
                nc.sync.dma_start(out=src_hi[t][64:128, s0:s1],
                                  in_=src_lo[t][0:64, s0:s1])
                nc.sync.dma_start(out=src_hi[t][0:64, s0:s1],
                                  in_=src_lo[t][64:128, s0:s1])

            def swap_block(src_lo, src_hi, t, half, nn):
                """DMA-swap one 512-col block (for just-in-time K pair 0)."""
                s0 = half * 1024 + nn * 512
                s1 = s0 + 512
                nc.sync.dma_start(out=src_hi[t][64:128, s0:s1],
                                  in_=src_lo[t][0:64, s0:s1])
                nc.sync.dma_start(out=src_hi[t][0:64, s0:s1],
                                  in_=src_lo[t][64:128, s0:s1])

            def v_fill(dst_ap):
                return dst_ap.rearrange("p (h t) -> p h t", t=DH + 1)

            def v_lead(k, m, pvt):
                """V chunk m into psV columns (lead-in only)."""
                mh, mo = divmod(m, 8)
                for d in range(NDT):
                    nc.tensor.matmul(
                        pvt[:, k * 512:(k + 1) * 512],
                        lhsT=xT_s[d][mh][:, mo * 128:(mo + 1) * 128],
                        rhs=wv_s[d][:, :],
                        start=(d == 0),
                        stop=(d == NDT - 1),
                    )
                va = v_fill(v_aug[m])
                nc.vector.tensor_copy(
                    va[:, :, 0:DH],
                    pvt[:, k * 512:(k + 1) * 512].rearrange(
                        "p (h t) -> p h t", t=DH),
                )
                nc.vector.memset(va[:, :, DH:DH + 1], 1.0)

            def v_piece(m):
                """4-matmul filler piece: V projection for j-chunk m."""
                pa = psA.tile([128, 1024], f32, name="pa", tag="pa")
                mh, mo = divmod(m, 8)
                for d in range(NDT):
                    nc.tensor.matmul(
                        pa[:, 0:512],
                        lhsT=xT_s[d][mh][:, mo * 128:(mo + 1) * 128],
                        rhs=wv_s[d][:, :],
                        start=(d == 0),
                        stop=(d == NDT - 1),
                    )
                va = v_fill(v_aug[m])
                nc.vector.tensor_copy(
                    va[:, :, 0:DH],
                    pa[:, 0:512].rearrange("p (h t) -> p h t", t=DH),
                )
                nc.vector.memset(va[:, :, DH:DH + 1], 1.0)

            def op01_piece(ch, hf):
                """Out-projection kt0+kt1 into PSUM, staged to SBUF bf16."""
                pa = psA.tile([128, 1024], f32, name="pa", tag="pa")
                for st2 in range(2):
                    st = hf * 2 + st2
                    for kt in range(2):
                        nc.tensor.matmul(
                            pa[:, st2 * 512:(st2 + 1) * 512],
                            lhsT=wo_s[kt][:, ch * 128:(ch + 1) * 128],
                            rhs=ot[kt][:, st * 512:(st + 1) * 512],
                            start=(kt == 0),
                            stop=(kt == 1),
                        )
                nc.vector.tensor_copy(stg[ch][hf][:, :], pa[:, :])

            def op2_piece(ch, hf):
                """Out-projection kt2 added into the bf16 stage."""
                pa = psA.tile([128, 1024], f32, name="pa", tag="pa")
                for st2 in range(2):
                    st = hf * 2 + st2
                    nc.tensor.matmul(
                        pa[:, st2 * 512:(st2 + 1) * 512],
                        lhsT=wo_s[2][:, ch * 128:(ch + 1) * 128],
                        rhs=ot[2][:, st * 512:(st + 1) * 512],
                    )
                nc.vector.scalar_tensor_tensor(
                    out=stg[ch][hf][:, :], in0=pa[:, :], scalar=0.0,
                    in1=stg[ch][hf][:, :], op0=Add, op1=Add)

            # ---- lead-in compute ----
            # Order: everything scores-jc0 needs first (Q pair-0 + swaps,
            # K block 0 + swap); V 0-3 last so their late wv DMA never
            # blocks the Q/K matmuls.
            pvt0 = psV.tile([128, 4 * 512], f32, name="pvt", tag="pv")
            qk_half("q", qt_lo, 0, 0)
            qk_half("q", qt_lo, 0, 1)
            swap_half(qt_lo, qt_hi, 0, 0)
            swap_half(qt_lo, qt_hi, 0, 1)
            qk_piece("k", kt_lo, 0, 0, 0)
            swap_block(kt_lo, kt_hi, 0, 0, 0)
            v_lead(0, 0, pvt0)
            v_lead(1, 1, pvt0)
            v_lead(2, 2, pvt0)
            v_lead(3, 3, pvt0)

            # ---- filler schedule: {head: {jc: [thunk, ...]}} ----
            fillers = {h: {} for h in range(H)}

            def add_filler(h, jc, fn):
                fillers[h].setdefault(jc, []).append(fn)

            # head 0: remaining K pair-0 blocks (JIT, block b feeds jc 4b),
            # V chunks 4-15 (chunk m consumed by PV(m) at loop-jc m+2).
            add_filler(0, 0, lambda: qk_piece("k", kt_lo, 0, 0, 1))
            add_filler(0, 0, lambda: swap_block(kt_lo, kt_hi, 0, 0, 1))
            add_filler(0, 1, lambda: v_piece(4))
            add_filler(0, 2, lambda: v_piece(5))
            add_filler(0, 3, lambda: qk_piece("k", kt_lo, 0, 1, 0))
            add_filler(0, 3, lambda: swap_block(kt_lo, kt_hi, 0, 1, 0))
            add_filler(0, 4, lambda: v_piece(6))
            add_filler(0, 5, lambda: v_piece(7))
            add_filler(0, 6, lambda: v_piece(8))
            add_filler(0, 7, lambda: qk_piece("k", kt_lo, 0, 1, 1))
            add_filler(0, 7, lambda: swap_block(kt_lo, kt_hi, 0, 1, 1))
            for i, m in enumerate(range(9, 16)):
                add_filler(0, 8 + i, (lambda m=m: v_piece(m)))

            # Q/K projection for pair t: pair 1 lives entirely in head 1
            # (head 0 is full of V); pairs 2-3 put Q late in head 2t-2 and
            # K (+ swaps) in head 2t-1. kt_hi sh1 deadline is head 2t jc8.
            def qk_sched(t, slots):
                for (hh, jc), fn in zip(slots, (
                    lambda: qk_piece("q", qt_lo, t, 0, 0),
                    lambda: qk_piece("q", qt_lo, t, 0, 1),
                    lambda: qk_piece("q", qt_lo, t, 1, 0),
                    lambda: qk_piece("q", qt_lo, t, 1, 1),
                    lambda: swap_half(qt_lo, qt_hi, t, 0),
                    lambda: swap_half(qt_lo, qt_hi, t, 1),
                    lambda: qk_piece("k", kt_lo, t, 0, 0),
                    lambda: qk_piece("k", kt_lo, t, 0, 1),
                    lambda: swap_half(kt_lo, kt_hi, t, 0),
                    lambda: qk_piece("k", kt_lo, t, 1, 0),
                    lambda: qk_piece("k", kt_lo, t, 1, 1),
                    lambda: swap_half(kt_lo, kt_hi, t, 1),
                )):
                    add_filler(hh, jc, fn)

            qk_sched(1, [(1, 0), (1, 1), (1, 2), (1, 3), (1, 4), (1, 5),
                         (1, 6), (1, 7), (1, 8), (1, 9), (1, 11), (1, 12)])
            for t in (2, 3):
                hq, hk = 2 * t - 2, 2 * t - 1
                qk_sched(t, [(hq, 12), (hq, 13), (hq, 14), (hq, 15),
                             (hk, 0), (hk, 1), (hk, 2), (hk, 3), (hk, 4),
                             (hk, 5), (hk, 7), (hk, 8)])

            # out-projection partials: kt0+kt1 in heads 5-6 (ot0/ot1 are
            # final after head 3's normalization), kt2 in head 7.
            add_filler(5, 10, lambda: op01_piece(0, 0))
            add_filler(5, 12, lambda: op01_piece(0, 1))
            add_filler(5, 14, lambda: op01_piece(1, 0))
            for i in range(5):
                ch, hf = divmod(i + 3, 2)
                add_filler(6, 2 * i + 1, (lambda ch=ch, hf=hf:
                                          op01_piece(ch, hf)))
            for i in range(8):
                ch, hf = divmod(i, 2)
                add_filler(7, 2 * i + 1, (lambda ch=ch, hf=hf:
                                          op2_piece(ch, hf)))

            # ---- attention, head by head ----
            pv_last = None
            TRAIL = 2
            for h in range(H):
                t, p = h // 2, h % 2
                lo_sl = slice(64 * p, 64 * p + 64)
                hi_sl = slice(64 * (1 - p), 64 * (1 - p) + 64)
                pv = psV.tile([128, 4 * 512], f32, name="pvh", tag="pv")
                ets = {}

                def pv_mms(jc):
                    for it in range(NST):
                        nc.tensor.matmul(
                            pv[0:DH + 1, it * 512:(it + 1) * 512],
                            lhsT=v_aug[jc][:, h * (DH + 1):(h + 1) * (DH + 1)],
                            rhs=ets[jc][:, it * 512:(it + 1) * 512],
                            start=(jc == 0),
                            stop=(jc == NSC - 1),
                        )

                for jc in range(NSC):
                    et = epool.tile([128, S], bf16, name="et", tag="et")
                    ets[jc] = et
                    for half in range(2):
                        pa = psA.tile([128, 1024], f32, name="pa", tag="pa")
                        i0, i1 = 2 * half, 2 * half + 1
                        nc.tensor.matmul(
                            pa[:, 0:512],
                            lhsT=kt_lo[t][lo_sl, jc * 128:(jc + 1) * 128],
                            rhs=qt_lo[t][lo_sl, i0 * 512:(i0 + 1) * 512],
                        )
                        nc.tensor.matmul(
                            pa[:, 512:1024],
                            lhsT=kt_hi[t][hi_sl, jc * 128:(jc + 1) * 128],
                            rhs=qt_hi[t][hi_sl, i1 * 512:(i1 + 1) * 512],
                        )
                        nc.scalar.activation(
                            out=et[:, half * 1024:(half + 1) * 1024],
                            in_=pa[:, :],
                            func=Exp,
                            scale=SCALE,
                        )
                    for fn in fillers[h].get(jc, ()):
                        fn()
                    if jc >= TRAIL:
                        pv_mms(jc - TRAIL)
                for jc in range(NSC - TRAIL, NSC):
                    pv_mms(jc)

                if h < H - 1:
                    # decouple normalization: copy O_un+den out of PSUM,
                    # reciprocal via (128,16) reshape, DRAM partition
                    # broadcast, multiply. All hidden under head h+1.
                    oun = spool.tile([DH + 1, S], f32, name="oun", tag="oun")
                    nc.vector.tensor_copy(oun[:, :], pv[0:DH + 1, :])
                    den128 = spool.tile([128, 16], f32, name="den128",
                                        tag="d128")
                    nc.sync.dma_start(out=den128[:, :], in_=oun[DH:DH + 1, :])
                    nc.vector.reciprocal(out=den128[:, :], in_=den128[:, :])
                    nc.sync.dma_start(out=den_dram[h, :], in_=den128[:, :])
                    bc = spool.tile([64, S], f32, name="bc", tag="bc")
                    dd = den_dram[h:h + 1, :]
                    bcast_src = bass.AP(
                        tensor=dd.tensor,
                        offset=dd.offset,
                        ap=[[0, 64]] + [list(x) for x in dd.ap[1:]],
                    )
                    nc.sync.dma_start(out=bc[:, :], in_=bcast_src)
                    nc.vector.tensor_mul(
                        ot[t][64 * p:64 * p + 64, :], oun[0:DH, :], bc[:, :])
                else:
                    pv_last = pv

            # ---- head 7 normalization (critical tail) ----
            # denominator row straight out of PSUM via DMA into a (128,16)
            # tile; reciprocal is then a ~16-cycle op instead of a
            # single-partition 2048-cycle crawl.
            row7 = spool.tile([DH + 1, S], f32, name="oun", tag="oun")
            nc.scalar.copy(row7[0:1, :], pv_last[DH:DH + 1, :])
            den7 = spool.tile([128, 16], f32, name="den128", tag="d128")
            nc.sync.dma_start(out=den7[:, :], in_=row7[0:1, :])
            nc.vector.reciprocal(out=den7[:, :], in_=den7[:, :])
            nc.sync.dma_start(out=den_dram[7, :], in_=den7[:, :])
            bc7 = spool.tile([64, S], f32, name="bc", tag="bc")
            dd7 = den_dram[7:8, :]
            bcast7 = bass.AP(
                tensor=dd7.tensor,
                offset=dd7.offset,
                ap=[[0, 64]] + [list(x) for x in dd7.ap[1:]],
            )
            nc.sync.dma_start(out=bc7[:, :], in_=bcast7)
            # keep the PE clock hot through the normalization bubble
            for k in range(8):
                ja = psA.tile([128, 1024], f32, name="pa", tag="pa")
                for half in range(2):
                    nc.tensor.matmul(
                        ja[:, half * 512:(half + 1) * 512],
                        lhsT=junk_sb[:, 0:128],
                        rhs=junk_sb[:, :],
                    )
            nc.vector.tensor_mul(
                ot[3][64:128, :], pv_last[0:DH, :], bc7[:, :])

            # ---- tail: kt3 matmuls + fused (po + bias) + stage combine ----
            for i in range(8):
                ch, hf = divmod(i, 2)
                po = psA.tile([128, 1024], f32, name="pa", tag="pa")
                for st2 in range(2):
                    st = hf * 2 + st2
                    nc.tensor.matmul(
                        po[:, st2 * 512:(st2 + 1) * 512],
                        lhsT=wo_s[3][:, ch * 128:(ch + 1) * 128],
                        rhs=ot[3][:, st * 512:(st + 1) * 512],
                    )
                ostage = opool.tile([128, 1024], f32, name="ost", tag="ost")
                eng = nc.vector
                eng.scalar_tensor_tensor(
                    out=ostage[:, :], in0=po[:, :], scalar=bo_s[ch][:, :],
                    in1=stg[ch][hf][:, :], op0=Add, op1=Add)
                nc.sync.dma_start(
                    out=out[ch * 128:(ch + 1) * 128,
                            hf * 1024:(hf + 1) * 1024],
                    in_=ostage[:, :],
                )

    nc.finalize()
    return nc


_NC_CACHE = None


def _get_nc():
    global _NC_CACHE
    if _NC_CACHE is None:
        _NC_CACHE = _build_kernel()
    return _NC_CACHE


def kernel(x, W_qkv, W_out, b_out):
    from concourse.bass_utils import run_bass_kernel_spmd

    bf16 = ml_dtypes.bfloat16

    # head-interleave and transpose the qkv weight: row 192h+{0,64,128}+c of
    # W_qkv is q/k/v row (h, c); regroup to e' = 64h+c and transpose to [d, e']
    w3 = W_qkv.reshape(H, 3, DH, D)
    wq_h = np.ascontiguousarray(w3[:, 0].reshape(INNER, D).T).astype(bf16)
    wk_h = np.ascontiguousarray(w3[:, 1].reshape(INNER, D).T).astype(bf16)
    wv_h = np.ascontiguousarray(w3[:, 2].reshape(INNER, D).T).astype(bf16)
    wo_h = np.ascontiguousarray(W_out.T).astype(bf16)  # [hc, d]
    bo_h = np.ascontiguousarray(b_out.reshape(NDT, 128, 1)).astype(np.float32)

    in_maps = []
    for b in range(N_CORES):
        xT_b = np.ascontiguousarray(x[b].T).astype(bf16)  # [d, s]
        in_maps.append({
            "xT": xT_b, "wq": wq_h, "wk": wk_h, "wv": wv_h,
            "wo": wo_h, "bo": bo_h,
        })

    nc = _get_nc()
    res = run_bass_kernel_spmd(nc, in_maps, list(range(N_CORES)))
    outs = [res.results[b]["out"].T for b in range(N_CORES)]  # [s, d] each
    return np.ascontiguousarray(np.stack(outs, axis=0)).astype(np.float32)


# revision 17
# speedup vs baseline: 1.0172x; 1.0172x over previous
"""Multi-head attention (B=8, S=2048, D=512, H=8, DH=64) on 8 TRN2 NeuronCores.

Data-parallel over batch; per core everything is transposed (feature on
partitions) so softmax reductions ride the TensorE contraction axis.

v2 schedule (trace-driven rework of the staged baseline):
  - Steady state is ACT-bound at ~1005ns per [128,1024] exp; PE runs
    ~830-940ns/half-jc, so filler projection work must be fine-grained.
  - PV trails the exp stream by TWO j-chunks (was 1), so PV matmuls never
    wait on the just-issued exp (-120ns/jc of PE stall).
  - Lead-in only computes what head 0 jc0 needs: Q pair-0 (full), K pair-0
    column block 0, V chunks 0-3; wq/wk pair-0 column slices are loaded as
    separate tiles so the first matmuls don't wait on the full weight DMA.
    K blocks 1-3, V 4-15 become early head-0 fillers with per-jc deadlines.
  - Output projection is split by contraction block: kt0+kt1 accumulate
    into PSUM during heads 5-6 and stage to SBUF (bf16), kt2 adds in during
    head 7, so the tail only runs kt3 matmuls + one fused
    (po + bias) + stage combine per output chunk (DVE/GpSimd split).
  - Head-7 normalization avoids the slow [1,2048] single-partition ops:
    DMA the PSUM denominator row into a [128,16] tile, reciprocal there,
    DRAM partition-broadcast back, multiply out of PSUM.
"""

import numpy as np
import ml_dtypes

B, S, D = 8, 2048, 512
H, DH = 8, 64
INNER = H * DH
SCALE = DH ** -0.5

N_CORES = 8
NDT = D // 128   # 4 contraction tiles
NSC = S // 128   # 16 s-chunks (j-chunks)
NST = S // 512   # 4 s-tiles


def _build_kernel():
    import concourse.bass as bass
    import concourse.mybir as mybir
    import concourse.tile as tile
    from concourse import bacc

    bf16 = mybir.dt.bfloat16
    f32 = mybir.dt.float32
    Exp = mybir.ActivationFunctionType.Exp
    Add = mybir.AluOpType.add

    nc = bacc.Bacc()

    xT = nc.declare_dram_parameter("xT", [D, S], bf16, isOutput=False)
    wq = nc.declare_dram_parameter("wq", [D, INNER], bf16, isOutput=False)
    wk = nc.declare_dram_parameter("wk", [D, INNER], bf16, isOutput=False)
    wv = nc.declare_dram_parameter("wv", [D, INNER], bf16, isOutput=False)
    wo = nc.declare_dram_parameter("wo", [INNER, D], bf16, isOutput=False)
    bo = nc.declare_dram_parameter("bo", [NDT, 128, 1], f32, isOutput=False)
    out = nc.declare_dram_parameter("out", [D, S], bf16, isOutput=True)
    den_dram = nc.dram_tensor("den_scratch", [H, S], f32)

    with tile.TileContext(nc) as tc:
        with (
            tc.tile_pool(name="weights", bufs=1) as wpool,
            tc.tile_pool(name="acts", bufs=1) as apool,
            tc.tile_pool(name="et", bufs=4) as epool,
            tc.tile_pool(name="small", bufs=2) as spool,
            tc.tile_pool(name="ostage", bufs=2) as opool,
            tc.tile_pool(name="psA", bufs=2, space="PSUM") as psA,
            tc.tile_pool(name="psV", bufs=1, space="PSUM") as psV,
        ):
            # ---- SBUF tiles ----
            # wq/wk pair-0 column slice gets its own tile so lead-in matmuls
            # depend only on the small early DMA, not the full weight load.
            wqA = [wpool.tile([128, 128], bf16, name=f"wqA{d}", tag=f"wqA{d}")
                   for d in range(NDT)]
            wkA = [wpool.tile([128, 128], bf16, name=f"wkA{d}", tag=f"wkA{d}")
                   for d in range(NDT)]
            wqB = [wpool.tile([128, INNER - 128], bf16, name=f"wqB{d}",
                              tag=f"wqB{d}") for d in range(NDT)]
            wkB = [wpool.tile([128, INNER - 128], bf16, name=f"wkB{d}",
                              tag=f"wkB{d}") for d in range(NDT)]
            xT_s = [[wpool.tile([128, S // 2], bf16, name=f"xT{d}_{hf}",
                                tag=f"xT{d}_{hf}") for hf in range(2)]
                    for d in range(NDT)]
            wv_s = [wpool.tile([128, INNER], bf16, name=f"wv{d}", tag=f"wv{d}")
                    for d in range(NDT)]
            wo_s = [wpool.tile([128, D], bf16, name=f"wo{d}", tag=f"wo{d}")
                    for d in range(NDT)]
            bo_s = [wpool.tile([128, 1], f32, name=f"bo{d}", tag=f"bo{d}")
                    for d in range(NDT)]

            def wsel(w_pair, t):
                """(tile list, col offset) for head-pair t's projection."""
                if t == 0:
                    return (wqA if w_pair == "q" else wkA), 0
                return (wqB if w_pair == "q" else wkB), (t - 1) * 128

            # ---- prioritized input DMA ----
            for d in range(NDT):
                sl = slice(d * 128, (d + 1) * 128)
                nc.sync.dma_start(out=wqA[d][:], in_=wq[sl, 0:128])
                nc.scalar.dma_start(out=wkA[d][:], in_=wk[sl, 0:128])
            for d in range(NDT):
                sl = slice(d * 128, (d + 1) * 128)
                nc.sync.dma_start(out=xT_s[d][0][:], in_=xT[sl, 0:S // 2])
                nc.scalar.dma_start(out=xT_s[d][1][:], in_=xT[sl, S // 2:])
            for d in range(NDT):
                sl = slice(d * 128, (d + 1) * 128)
                nc.gpsimd.dma_start(out=wv_s[d][:], in_=wv[sl, :])
            for d in range(NDT):
                sl = slice(d * 128, (d + 1) * 128)
                nc.gpsimd.dma_start(out=wo_s[d][:], in_=wo[sl, :])
                nc.gpsimd.dma_start(out=bo_s[d][:], in_=bo[d, :, :])
            for d in range(NDT):
                sl = slice(d * 128, (d + 1) * 128)
                nc.sync.dma_start(out=wqB[d][:], in_=wq[sl, 128:])
                nc.scalar.dma_start(out=wkB[d][:], in_=wk[sl, 128:])

            # ---- QKV target tiles ----
            qt_lo = [apool.tile([128, S], bf16, name=f"qlo{t}", tag=f"qlo{t}")
                     for t in range(NDT)]
            kt_lo = [apool.tile([128, S], bf16, name=f"klo{t}", tag=f"klo{t}")
                     for t in range(NDT)]
            qt_hi = [apool.tile([128, S], bf16, name=f"qhi{t}", tag=f"qhi{t}")
                     for t in range(NDT)]
            kt_hi = [apool.tile([128, S], bf16, name=f"khi{t}", tag=f"khi{t}")
                     for t in range(NDT)]
            v_aug = [apool.tile([128, H * (DH + 1)], bf16, name=f"va{m}",
                                tag=f"va{m}") for m in range(NSC)]
            ot = [apool.tile([128, S], bf16, name=f"ot{t}", tag=f"ot{t}")
                  for t in range(NDT)]
            # out-projection kt{0,1}+kt2 partial sums, bf16 staging
            stg = [[wpool.tile([128, 1024], bf16, name=f"stg{ch}_{hf}",
                               tag=f"stg{ch}_{hf}") for hf in range(2)]
                   for ch in range(NDT)]

            # PE warm-up + GpSimd ucode preload (lazy LOAD_LIB is ~7us)
            junk_sb = wpool.tile([128, 512], bf16, name="junk", tag="junk")
            nc.vector.memset(junk_sb[:, :], 0.0)
            pre_bc = wpool.tile([2, 16], f32, name="prebc", tag="prebc")
            nc.vector.memset(pre_bc[:, :], 1.0)
            nc.gpsimd.partition_broadcast(pre_bc[:, :], pre_bc[0:1, :],
                                          channels=2)
            junk_ps = psV.tile([128, 4 * 512], f32, name="junkps", tag="pv")
            _junk_k = [0]

            def junk_fill(n, wide=True):
                """n matmuls into the junk PSUM tile to keep the HAM clock
                ramp alive across lead-in DMA waits."""
                for _ in range(n):
                    k = _junk_k[0] = _junk_k[0] + 1
                    nc.tensor.matmul(
                        junk_ps[:, (k % 4) * 512:(k % 4 + 1) * 512],
                        lhsT=junk_sb[:, 0:128],
                        rhs=junk_sb[:, 0:512] if wide else junk_sb[:, 0:128],
                    )

            junk_fill(8)

            # ---- projection piece helpers ----
            def qk_half(w_pair, dst, t, half):
                """One s-half (1024 cols) of a Q/K projection: 8 matmuls."""
                w_s, co = wsel(w_pair, t)
                pa = psA.tile([128, 1024], f32, name="pa", tag="pa")
                for nn in range(2):
                    s0 = nn * 512
                    for d in range(NDT):
                        nc.tensor.matmul(
                            pa[:, nn * 512:(nn + 1) * 512],
                            lhsT=w_s[d][:, co:co + 128],
                            rhs=xT_s[d][half][:, s0:s0 + 512],
                            start=(d == 0),
                            stop=(d == NDT - 1),
                        )
                nc.vector.tensor_copy(
                    dst[t][:, half * 1024:(half + 1) * 1024], pa[:, :])

            def qk_piece(w_pair, dst, t, half, nn):
                """4-matmul piece: one 512-col block of a Q/K projection."""
                w_s, co = wsel(w_pair, t)
                pa = psA.tile([128, 1024], f32, name="pa", tag="pa")
                s0 = nn * 512
                for d in range(NDT):
                    nc.tensor.matmul(
                        pa[:, 0:512],
                        lhsT=w_s[d][:, co:co + 128],
                        rhs=xT_s[d][half][:, s0:s0 + 512],
                        start=(d == 0),
                        stop=(d == NDT - 1),
                    )
                off = half * 1024 + nn * 512
                nc.vector.tensor_copy(dst[t][:, off:off + 512], pa[:, 0:512])

            def swap_half(src_lo, src_hi, t, sh):
                """DMA-swap one 1024-wide s-half of the lo replica into hi."""
                s0, s1 = sh * 1024, (sh + 1) * 1024
                nc.sync.dma_start(out=src_hi[t][64:128, s0:s1],
                                  in_=src_lo[t][0:64, s0:s1])
                nc.sync.dma_start(out=src_hi[t][0:64, s0:s1],
                                  in_=src_lo[t][64:128, s0:s1])

            def swap_block(src_lo, src_hi, t, half, nn):
                """DMA-swap one 512-col block (for just-in-time K pair 0)."""
                s0 = half * 1024 + nn * 512
                s1 = s0 + 512
                nc.sync.dma_start(out=src_hi[t][64:128, s0:s1],
                                  in_=src_lo[t][0:64, s0:s1])
                nc.sync.dma_start(out=src_hi[t][0:64, s0:s1],
                                  in_=src_lo[t][64:128, s0:s1])

            def v_fill(dst_ap):
                return dst_ap.rearrange("p (h t) -> p h t", t=DH + 1)

            def v_piece(m):
                """4-matmul filler piece: V projection for j-chunk m."""
                pa = psA.tile([128, 1024], f32, name="pa", tag="pa")
                mh, mo = divmod(m, 8)
                for d in range(NDT):
                    nc.tensor.matmul(
                        pa[:, 0:512],
                        lhsT=xT_s[d][mh][:, mo * 128:(mo + 1) * 128],
                        rhs=wv_s[d][:, :],
                        start=(d == 0),
                        stop=(d == NDT - 1),
                    )
                va = v_fill(v_aug[m])
                nc.vector.tensor_copy(
                    va[:, :, 0:DH],
                    pa[:, 0:512].rearrange("p (h t) -> p h t", t=DH),
                )
                nc.vector.memset(va[:, :, DH:DH + 1], 1.0)

            def op01_piece(ch, hf):
                """Out-projection kt0+kt1 into PSUM, staged to SBUF bf16."""
                pa = psA.tile([128, 1024], f32, name="pa", tag="pa")
                for st2 in range(2):
                    st = hf * 2 + st2
                    for kt in range(2):
                        nc.tensor.matmul(
                            pa[:, st2 * 512:(st2 + 1) * 512],
                            lhsT=wo_s[kt][:, ch * 128:(ch + 1) * 128],
                            rhs=ot[kt][:, st * 512:(st + 1) * 512],
                            start=(kt == 0),
                            stop=(kt == 1),
                        )
                nc.vector.tensor_copy(stg[ch][hf][:, :], pa[:, :])

            # ---- lead-in compute ----
            # Only what scores-jc0 needs (Q pair-0 + swaps, K block 0 +
            # swap); junk matmuls keep the clock ramp alive across DMA
            # waits so the real matmuls run at full pstate.
            qk_half("q", qt_lo, 0, 0)
            junk_fill(3)
            qk_half("q", qt_lo, 0, 1)
            swap_half(qt_lo, qt_hi, 0, 0)
            swap_half(qt_lo, qt_hi, 0, 1)
            junk_fill(3)
            qk_piece("k", kt_lo, 0, 0, 0)
            swap_block(kt_lo, kt_hi, 0, 0, 0)

            # ---- filler schedule: {head: {jc: [thunk, ...]}} ----
            fillers = {h: {} for h in range(H)}

            def add_filler(h, jc, fn):
                fillers[h].setdefault(jc, []).append(fn)

            # head 0: all 16 V chunks (chunk m consumed by PV(m) at loop-jc
            # m+2, and fillers run before that jc's PV — so V m at jc m is
            # two slots ahead) plus the remaining K pair-0 blocks (JIT,
            # block b feeds scores at jc 4b).
            for m in range(NSC):
                add_filler(0, m, (lambda m=m: v_piece(m)))
            add_filler(0, 1, lambda: qk_piece("k", kt_lo, 0, 0, 1))
            add_filler(0, 1, lambda: swap_block(kt_lo, kt_hi, 0, 0, 1))
            add_filler(0, 5, lambda: qk_piece("k", kt_lo, 0, 1, 0))
            add_filler(0, 5, lambda: swap_block(kt_lo, kt_hi, 0, 1, 0))
            add_filler(0, 8, lambda: qk_piece("k", kt_lo, 0, 1, 1))
            add_filler(0, 8, lambda: swap_block(kt_lo, kt_hi, 0, 1, 1))

            # Q/K projection for pair t: pair 1 lives entirely in head 1
            # (head 0 is full of V); pairs 2-3 put Q late in head 2t-2 and
            # K (+ swaps) in head 2t-1. kt_hi sh1 deadline is head 2t jc8.
            def qk_sched(t, slots):
                for (hh, jc), fn in zip(slots, (
                    lambda: qk_piece("q", qt_lo, t, 0, 0),
                    lambda: qk_piece("q", qt_lo, t, 0, 1),
                    lambda: qk_piece("q", qt_lo, t, 1, 0),
                    lambda: qk_piece("q", qt_lo, t, 1, 1),
                    lambda: swap_half(qt_lo, qt_hi, t, 0),
                    lambda: swap_half(qt_lo, qt_hi, t, 1),
                    lambda: qk_piece("k", kt_lo, t, 0, 0),
                    lambda: qk_piece("k", kt_lo, t, 0, 1),
                    lambda: swap_half(kt_lo, kt_hi, t, 0),
                    lambda: qk_piece("k", kt_lo, t, 1, 0),
                    lambda: qk_piece("k", kt_lo, t, 1, 1),
                    lambda: swap_half(kt_lo, kt_hi, t, 1),
                )):
                    add_filler(hh, jc, fn)

            qk_sched(1, [(1, 0), (1, 1), (1, 2), (1, 3), (1, 4), (1, 5),
                         (1, 6), (1, 7), (1, 8), (1, 9), (1, 11), (1, 12)])
            for t in (2, 3):
                hq, hk = 2 * t - 2, 2 * t - 1
                qk_sched(t, [(hq, 12), (hq, 13), (hq, 14), (hq, 15),
                             (hk, 0), (hk, 1), (hk, 2), (hk, 3), (hk, 4),
                             (hk, 5), (hk, 7), (hk, 8)])

            # out-projection partials: kt0+kt1 in heads 5-6 (ot0/ot1 are
            # final after head 3's normalization), kt2 in head 7.
            add_filler(5, 10, lambda: op01_piece(0, 0))
            add_filler(5, 12, lambda: op01_piece(0, 1))
            add_filler(5, 14, lambda: op01_piece(1, 0))
            for i in range(5):
                ch, hf = divmod(i + 3, 2)
                add_filler(6, 2 * i + 1, (lambda ch=ch, hf=hf:
                                          op01_piece(ch, hf)))

            # ---- attention, head by head ----
            pv_last = None
            TRAIL = 2
            for h in range(H):
                t, p = h // 2, h % 2
                lo_sl = slice(64 * p, 64 * p + 64)
                hi_sl = slice(64 * (1 - p), 64 * (1 - p) + 64)
                pv = psV.tile([128, 4 * 512], f32, name="pvh", tag="pv")
                ets = {}

                def pv_mms(jc):
                    for it in range(NST):
                        nc.tensor.matmul(
                            pv[0:DH + 1, it * 512:(it + 1) * 512],
                            lhsT=v_aug[jc][:, h * (DH + 1):(h + 1) * (DH + 1)],
                            rhs=ets[jc][:, it * 512:(it + 1) * 512],
                            start=(jc == 0),
                            stop=(jc == NSC - 1),
                        )

                for jc in range(NSC):
                    et = epool.tile([128, S], bf16, name="et", tag="et")
                    ets[jc] = et
                    for half in range(2):
                        pa = psA.tile([128, 1024], f32, name="pa", tag="pa")
                        i0, i1 = 2 * half, 2 * half + 1
                        nc.tensor.matmul(
                            pa[:, 0:512],
                            lhsT=kt_lo[t][lo_sl, jc * 128:(jc + 1) * 128],
                            rhs=qt_lo[t][lo_sl, i0 * 512:(i0 + 1) * 512],
                        )
                        nc.tensor.matmul(
                            pa[:, 512:1024],
                            lhsT=kt_hi[t][hi_sl, jc * 128:(jc + 1) * 128],
                            rhs=qt_hi[t][hi_sl, i1 * 512:(i1 + 1) * 512],
                        )
                        nc.scalar.activation(
                            out=et[:, half * 1024:(half + 1) * 1024],
                            in_=pa[:, :],
                            func=Exp,
                            scale=SCALE,
                        )
                    for fn in fillers[h].get(jc, ()):
                        fn()
                    if jc >= TRAIL:
                        pv_mms(jc - TRAIL)
                for jc in range(NSC - TRAIL, NSC):
                    pv_mms(jc)

                if h < H - 1:
                    # decouple normalization: copy O_un+den out of PSUM,
                    # reciprocal via (128,16) reshape, DRAM partition
                    # broadcast, multiply. All hidden under head h+1.
                    oun = spool.tile([DH + 1, S], f32, name="oun", tag="oun")
                    nc.vector.tensor_copy(oun[:, :], pv[0:DH + 1, :])
                    den128 = spool.tile([128, 16], f32, name="den128",
                                        tag="d128")
                    nc.sync.dma_start(out=den128[:, :], in_=oun[DH:DH + 1, :])
                    nc.vector.reciprocal(out=den128[:, :], in_=den128[:, :])
                    nc.sync.dma_start(out=den_dram[h, :], in_=den128[:, :])
                    bc = spool.tile([64, S], f32, name="bc", tag="bc")
                    dd = den_dram[h:h + 1, :]
                    bcast_src = bass.AP(
                        tensor=dd.tensor,
                        offset=dd.offset,
                        ap=[[0, 64]] + [list(x) for x in dd.ap[1:]],
                    )
                    nc.sync.dma_start(out=bc[:, :], in_=bcast_src)
                    nc.vector.tensor_mul(
                        ot[t][64 * p:64 * p + 64, :], oun[0:DH, :], bc[:, :])
                else:
                    pv_last = pv

            # ---- head 7 normalization (critical tail) ----
            # ACT copies the PSUM denominator row out (ACT is idle now),
            # DVE fast-reciprocal on the row, GpSimd partition-broadcast
            # (measured 3.3us — the DRAM round trip stalls ~7us), then
            # block-wise multiplies so the kt matmuls can chase them.
            row7 = spool.tile([DH + 1, S], f32, name="oun", tag="oun")
            nc.scalar.copy(row7[0:1, :], pv_last[DH:DH + 1, :])
            rec7 = spool.tile([DH + 1, S], f32, name="oun", tag="oun")
            nc.vector.reciprocal_approx_fast(out=rec7[0:1, :],
                                             in_=row7[0:1, :])
            bc7 = spool.tile([64, S], f32, name="bc", tag="bc")
            nc.gpsimd.partition_broadcast(bc7[:, :], rec7[0:1, :],
                                          channels=64)
            # keep the PE clock hot through the normalization bubble
            for k in range(10):
                ja = psA.tile([128, 1024], f32, name="pa", tag="pa")
                for half in range(2):
                    nc.tensor.matmul(
                        ja[:, half * 512:(half + 1) * 512],
                        lhsT=junk_sb[:, 0:128],
                        rhs=junk_sb[:, :],
                    )
            for st in range(NST):
                sl = slice(st * 512, (st + 1) * 512)
                nc.vector.tensor_mul(
                    ot[3][64:128, sl], pv_last[0:DH, sl], bc7[:, sl])

            # ---- tail: kt2+kt3 matmuls + (po + bias) + stage combine ----
            # hf-major so half-0 chunks start right after the first two
            # normalize blocks; combines split DVE / ACT+GpSimd to overlap.
            for i in range(8):
                hf, ch = divmod(i, 4)
                po = psA.tile([128, 1024], f32, name="pa", tag="pa")
                for st2 in range(2):
                    st = hf * 2 + st2
                    for kt in (2, 3):
                        nc.tensor.matmul(
                            po[:, st2 * 512:(st2 + 1) * 512],
                            lhsT=wo_s[kt][:, ch * 128:(ch + 1) * 128],
                            rhs=ot[kt][:, st * 512:(st + 1) * 512],
                            start=(kt == 2),
                            stop=(kt == 3),
                        )
                ostage = opool.tile([128, 1024], bf16, name="ost", tag="ost")
                if i in (1, 3, 5):
                    tmp = opool.tile([128, 1024], bf16, name="tmp", tag="tmp")
                    nc.scalar.add(tmp[:, :], po[:, :], bo_s[ch][:, :])
                    nc.gpsimd.tensor_add(ostage[:, :], tmp[:, :],
                                         stg[ch][hf][:, :])
                else:
                    nc.vector.scalar_tensor_tensor(
                        out=ostage[:, :], in0=po[:, :],
                        scalar=bo_s[ch][:, :],
                        in1=stg[ch][hf][:, :], op0=Add, op1=Add)
                nc.sync.dma_start(
                    out=out[ch * 128:(ch + 1) * 128,
                            hf * 1024:(hf + 1) * 1024],
                    in_=ostage[:, :],
                )

    nc.finalize()
    return nc


_NC_CACHE = None


def _get_nc():
    global _NC_CACHE
    if _NC_CACHE is None:
        _NC_CACHE = _build_kernel()
    return _NC_CACHE


def kernel(x, W_qkv, W_out, b_out):
    from concourse.bass_utils import run_bass_kernel_spmd

    bf16 = ml_dtypes.bfloat16

    # head-interleave and transpose the qkv weight: row 192h+{0,64,128}+c of
    # W_qkv is q/k/v row (h, c); regroup to e' = 64h+c and transpose to [d, e']
    w3 = W_qkv.reshape(H, 3, DH, D)
    wq_h = np.ascontiguousarray(w3[:, 0].reshape(INNER, D).T).astype(bf16)
    wk_h = np.ascontiguousarray(w3[:, 1].reshape(INNER, D).T).astype(bf16)
    wv_h = np.ascontiguousarray(w3[:, 2].reshape(INNER, D).T).astype(bf16)
    wo_h = np.ascontiguousarray(W_out.T).astype(bf16)  # [hc, d]
    bo_h = np.ascontiguousarray(b_out.reshape(NDT, 128, 1)).astype(np.float32)

    in_maps = []
    for b in range(N_CORES):
        xT_b = np.ascontiguousarray(x[b].T).astype(bf16)  # [d, s]
        in_maps.append({
            "xT": xT_b, "wq": wq_h, "wk": wk_h, "wv": wv_h,
            "wo": wo_h, "bo": bo_h,
        })

    nc = _get_nc()
    res = run_bass_kernel_spmd(nc, in_maps, list(range(N_CORES)))
    outs = [res.results[b]["out"].T for b in range(N_CORES)]  # [s, d] each
    return np.ascontiguousarray(np.stack(outs, axis=0)).astype(np.float32)


# revision 24
# speedup vs baseline: 1.0243x; 1.0069x over previous
"""Multi-head attention (B=8, S=2048, D=512, H=8, DH=64) on 8 TRN2 NeuronCores.

Data-parallel over batch; per core everything is transposed (feature on
partitions) so softmax reductions ride the TensorE contraction axis.

v2 schedule (trace-driven rework of the staged baseline):
  - Steady state is ACT-bound at ~1005ns per [128,1024] exp; PE runs
    ~830-940ns/half-jc, so filler projection work must be fine-grained.
  - PV trails the exp stream by TWO j-chunks (was 1), so PV matmuls never
    wait on the just-issued exp (-120ns/jc of PE stall).
  - Lead-in only computes what head 0 jc0 needs: Q pair-0 (full), K pair-0
    column block 0, V chunks 0-3; wq/wk pair-0 column slices are loaded as
    separate tiles so the first matmuls don't wait on the full weight DMA.
    K blocks 1-3, V 4-15 become early head-0 fillers with per-jc deadlines.
  - Output projection is split by contraction block: kt0+kt1 accumulate
    into PSUM during heads 5-6 and stage to SBUF (bf16), kt2 adds in during
    head 7, so the tail only runs kt3 matmuls + one fused
    (po + bias) + stage combine per output chunk (DVE/GpSimd split).
  - Head-7 normalization avoids the slow [1,2048] single-partition ops:
    DMA the PSUM denominator row into a [128,16] tile, reciprocal there,
    DRAM partition-broadcast back, multiply out of PSUM.
"""

import numpy as np
import ml_dtypes

B, S, D = 8, 2048, 512
H, DH = 8, 64
INNER = H * DH
SCALE = DH ** -0.5

N_CORES = 8
NDT = D // 128   # 4 contraction tiles
NSC = S // 128   # 16 s-chunks (j-chunks)
NST = S // 512   # 4 s-tiles


def _build_kernel():
    import concourse.bass as bass
    import concourse.mybir as mybir
    import concourse.tile as tile
    from concourse import bacc

    bf16 = mybir.dt.bfloat16
    f32 = mybir.dt.float32
    Exp = mybir.ActivationFunctionType.Exp
    Add = mybir.AluOpType.add

    nc = bacc.Bacc()

    xT = nc.declare_dram_parameter("xT", [D, S], bf16, isOutput=False)
    wq = nc.declare_dram_parameter("wq", [D, INNER], bf16, isOutput=False)
    wk = nc.declare_dram_parameter("wk", [D, INNER], bf16, isOutput=False)
    wv = nc.declare_dram_parameter("wv", [D, INNER], bf16, isOutput=False)
    wo = nc.declare_dram_parameter("wo", [INNER, D], bf16, isOutput=False)
    bo = nc.declare_dram_parameter("bo", [NDT, 128, 1], f32, isOutput=False)
    out = nc.declare_dram_parameter("out", [D, S], bf16, isOutput=True)
    den_dram = nc.dram_tensor("den_scratch", [H, S], f32)

    with tile.TileContext(nc) as tc:
        with (
            tc.tile_pool(name="weights", bufs=1) as wpool,
            tc.tile_pool(name="acts", bufs=1) as apool,
            tc.tile_pool(name="et", bufs=4) as epool,
            tc.tile_pool(name="small", bufs=2) as spool,
            tc.tile_pool(name="ostage", bufs=2) as opool,
            tc.tile_pool(name="psA", bufs=2, space="PSUM") as psA,
            tc.tile_pool(name="psV", bufs=1, space="PSUM") as psV,
        ):
            # ---- SBUF tiles ----
            # wq/wk pair-0 column slice gets its own tile so lead-in matmuls
            # depend only on the small early DMA, not the full weight load.
            wqA = [wpool.tile([128, 128], bf16, name=f"wqA{d}", tag=f"wqA{d}")
                   for d in range(NDT)]
            wkA = [wpool.tile([128, 128], bf16, name=f"wkA{d}", tag=f"wkA{d}")
                   for d in range(NDT)]
            wqB = [wpool.tile([128, INNER - 128], bf16, name=f"wqB{d}",
                              tag=f"wqB{d}") for d in range(NDT)]
            wkB = [wpool.tile([128, INNER - 128], bf16, name=f"wkB{d}",
                              tag=f"wkB{d}") for d in range(NDT)]
            xT_s = [[wpool.tile([128, S // 2], bf16, name=f"xT{d}_{hf}",
                                tag=f"xT{d}_{hf}") for hf in range(2)]
                    for d in range(NDT)]
            wv_s = [wpool.tile([128, INNER], bf16, name=f"wv{d}", tag=f"wv{d}")
                    for d in range(NDT)]
            wo_s = [wpool.tile([128, D], bf16, name=f"wo{d}", tag=f"wo{d}")
                    for d in range(NDT)]
            bo_s = [wpool.tile([128, 1], f32, name=f"bo{d}", tag=f"bo{d}")
                    for d in range(NDT)]

            def wsel(w_pair, t):
                """(tile list, col offset) for head-pair t's projection."""
                if t == 0:
                    return (wqA if w_pair == "q" else wkA), 0
                return (wqB if w_pair == "q" else wkB), (t - 1) * 128

            # ---- prioritized input DMA ----
            # xT first (it gates the whole Q0 chain), split across the
            # sync/scalar issue queues; nothing on gpsimd (descriptor gen
            # there costs ~630ns per dma_start and delays everything).
            for d in range(NDT):
                sl = slice(d * 128, (d + 1) * 128)
                nc.sync.dma_start(out=xT_s[d][0][:], in_=xT[sl, 0:S // 2])
                nc.scalar.dma_start(out=xT_s[d][1][:], in_=xT[sl, S // 2:])
            for d in range(NDT):
                sl = slice(d * 128, (d + 1) * 128)
                nc.sync.dma_start(out=wqA[d][:], in_=wq[sl, 0:128])
                nc.scalar.dma_start(out=wkA[d][:], in_=wk[sl, 0:128])
            for d in range(NDT):
                sl = slice(d * 128, (d + 1) * 128)
                nc.sync.dma_start(out=wv_s[d][:], in_=wv[sl, :])
                nc.scalar.dma_start(out=wo_s[d][:], in_=wo[sl, :])
                nc.scalar.dma_start(out=bo_s[d][:], in_=bo[d, :, :])
            for d in range(NDT):
                sl = slice(d * 128, (d + 1) * 128)
                nc.sync.dma_start(out=wqB[d][:], in_=wq[sl, 128:])
                nc.scalar.dma_start(out=wkB[d][:], in_=wk[sl, 128:])

            # ---- QKV target tiles ----
            qt_lo = [apool.tile([128, S], bf16, name=f"qlo{t}", tag=f"qlo{t}")
                     for t in range(NDT)]
            kt_lo = [apool.tile([128, S], bf16, name=f"klo{t}", tag=f"klo{t}")
                     for t in range(NDT)]
            qt_hi = [apool.tile([128, S], bf16, name=f"qhi{t}", tag=f"qhi{t}")
                     for t in range(NDT)]
            kt_hi = [apool.tile([128, S], bf16, name=f"khi{t}", tag=f"khi{t}")
                     for t in range(NDT)]
            v_aug = [apool.tile([128, H * (DH + 1)], bf16, name=f"va{m}",
                                tag=f"va{m}") for m in range(NSC)]
            ot = [apool.tile([128, S], bf16, name=f"ot{t}", tag=f"ot{t}")
                  for t in range(NDT)]
            # out-projection kt{0,1}+kt2 partial sums, bf16 staging
            stg = [[wpool.tile([128, 1024], bf16, name=f"stg{ch}_{hf}",
                               tag=f"stg{ch}_{hf}") for hf in range(2)]
                   for ch in range(NDT)]

            # PE warm-up + GpSimd ucode preload (lazy LOAD_LIB is ~7us)
            junk_sb = wpool.tile([128, 512], bf16, name="junk", tag="junk")
            nc.vector.memset(junk_sb[:, :], 0.0)
            pre_bc = wpool.tile([64, 16], f32, name="prebc", tag="prebc")
            nc.vector.memset(pre_bc[:, :], 1.0)
            nc.gpsimd.partition_broadcast(pre_bc[:, :], pre_bc[0:1, :],
                                          channels=64)
            junk_ps = psV.tile([128, 4 * 512], f32, name="junkps", tag="pv")
            _junk_k = [0]

            def junk_fill(n, wide=True):
                """n matmuls into the junk PSUM tile to keep the HAM clock
                ramp alive across lead-in DMA waits."""
                for _ in range(n):
                    k = _junk_k[0] = _junk_k[0] + 1
                    nc.tensor.matmul(
                        junk_ps[:, (k % 4) * 512:(k % 4 + 1) * 512],
                        lhsT=junk_sb[:, 0:128],
                        rhs=junk_sb[:, 0:512] if wide else junk_sb[:, 0:128],
                    )

            junk_fill(12)

            # ---- projection piece helpers ----
            def qk_half(w_pair, dst, t, half):
                """One s-half (1024 cols) of a Q/K projection: 8 matmuls."""
                w_s, co = wsel(w_pair, t)
                pa = psA.tile([128, 1024], f32, name="pa", tag="pa")
                for nn in range(2):
                    s0 = nn * 512
                    for d in range(NDT):
                        nc.tensor.matmul(
                            pa[:, nn * 512:(nn + 1) * 512],
                            lhsT=w_s[d][:, co:co + 128],
                            rhs=xT_s[d][half][:, s0:s0 + 512],
                            start=(d == 0),
                            stop=(d == NDT - 1),
                        )
                nc.vector.tensor_copy(
                    dst[t][:, half * 1024:(half + 1) * 1024], pa[:, :])

            def qk_piece(w_pair, dst, t, half, nn):
                """4-matmul piece: one 512-col block of a Q/K projection."""
                w_s, co = wsel(w_pair, t)
                pa = psA.tile([128, 1024], f32, name="pa", tag="pa")
                s0 = nn * 512
                for d in range(NDT):
                    nc.tensor.matmul(
                        pa[:, 0:512],
                        lhsT=w_s[d][:, co:co + 128],
                        rhs=xT_s[d][half][:, s0:s0 + 512],
                        start=(d == 0),
                        stop=(d == NDT - 1),
                    )
                off = half * 1024 + nn * 512
                nc.vector.tensor_copy(dst[t][:, off:off + 512], pa[:, 0:512])

            def swap_half(src_lo, src_hi, t, sh):
                """DMA-swap one 1024-wide s-half of the lo replica into hi."""
                s0, s1 = sh * 1024, (sh + 1) * 1024
                nc.sync.dma_start(out=src_hi[t][64:128, s0:s1],
                                  in_=src_lo[t][0:64, s0:s1])
                nc.sync.dma_start(out=src_hi[t][0:64, s0:s1],
                                  in_=src_lo[t][64:128, s0:s1])

            def swap_block(src_lo, src_hi, t, half, nn):
                """DMA-swap one 512-col block (for just-in-time K pair 0)."""
                s0 = half * 1024 + nn * 512
                s1 = s0 + 512
                nc.sync.dma_start(out=src_hi[t][64:128, s0:s1],
                                  in_=src_lo[t][0:64, s0:s1])
                nc.sync.dma_start(out=src_hi[t][0:64, s0:s1],
                                  in_=src_lo[t][64:128, s0:s1])

            def v_fill(dst_ap):
                return dst_ap.rearrange("p (h t) -> p h t", t=DH + 1)

            def v_piece(m):
                """4-matmul filler piece: V projection for j-chunk m."""
                pa = psA.tile([128, 1024], f32, name="pa", tag="pa")
                mh, mo = divmod(m, 8)
                for d in range(NDT):
                    nc.tensor.matmul(
                        pa[:, 0:512],
                        lhsT=xT_s[d][mh][:, mo * 128:(mo + 1) * 128],
                        rhs=wv_s[d][:, :],
                        start=(d == 0),
                        stop=(d == NDT - 1),
                    )
                va = v_fill(v_aug[m])
                nc.vector.tensor_copy(
                    va[:, :, 0:DH],
                    pa[:, 0:512].rearrange("p (h t) -> p h t", t=DH),
                )
                nc.vector.memset(va[:, :, DH:DH + 1], 1.0)

            def op01_piece(ch, hf):
                """Out-projection kt0+kt1 into PSUM, staged to SBUF bf16."""
                pa = psA.tile([128, 1024], f32, name="pa", tag="pa")
                for st2 in range(2):
                    st = hf * 2 + st2
                    for kt in range(2):
                        nc.tensor.matmul(
                            pa[:, st2 * 512:(st2 + 1) * 512],
                            lhsT=wo_s[kt][:, ch * 128:(ch + 1) * 128],
                            rhs=ot[kt][:, st * 512:(st + 1) * 512],
                            start=(kt == 0),
                            stop=(kt == 1),
                        )
                nc.vector.tensor_copy(stg[ch][hf][:, :], pa[:, :])

            # ---- lead-in compute ----
            # Only what scores-jc0 needs (Q pair-0 + swaps, K block 0 +
            # swap); junk matmuls keep the clock ramp alive across DMA
            # waits so the real matmuls run at full pstate.
            qk_half("q", qt_lo, 0, 0)
            junk_fill(3)
            qk_half("q", qt_lo, 0, 1)
            swap_half(qt_lo, qt_hi, 0, 0)
            swap_half(qt_lo, qt_hi, 0, 1)
            junk_fill(3)
            qk_piece("k", kt_lo, 0, 0, 0)
            swap_block(kt_lo, kt_hi, 0, 0, 0)

            # ---- filler schedule ----
            # Fillers that allocate a psA tile MUST come in pairs per jc
            # (one before PV, one after): an odd allocation count flips the
            # psA rotation parity and puts the next jc's scores in WAR with
            # the LATE exp (measured ~727ns ACT stall per lone piece vs
            # ~370ns for a paired one). DMA-only fillers (swaps) are free.
            fillers = {h: {} for h in range(H)}   # h -> jc -> [psA pieces]
            dfillers = {h: {} for h in range(H)}  # h -> jc -> [DMA thunks]

            def add_filler(h, jc, fn):
                fillers[h].setdefault(jc, []).append(fn)

            def add_dfiller(h, jc, fn):
                dfillers[h].setdefault(jc, []).append(fn)

            # head 0: all 16 V chunks as pairs at even jcs (chunk m is
            # consumed by PV(m) at loop-jc m+2), K pair-0 blocks at odd jcs.
            for m in range(0, NSC, 2):
                add_filler(0, m, (lambda m=m: v_piece(m)))
                add_filler(0, m, (lambda m=m: v_piece(m + 1)))
            add_filler(0, 1, lambda: qk_piece("k", kt_lo, 0, 0, 1))
            add_filler(0, 1, lambda: qk_piece("k", kt_lo, 0, 1, 0))
            add_dfiller(0, 1, lambda: swap_block(kt_lo, kt_hi, 0, 0, 1))
            add_dfiller(0, 1, lambda: swap_block(kt_lo, kt_hi, 0, 1, 0))
            add_filler(0, 5, lambda: qk_piece("k", kt_lo, 0, 1, 1))
            add_dfiller(0, 5, lambda: swap_block(kt_lo, kt_hi, 0, 1, 1))

            # Q/K projection for pair t: all 8 pieces as 4 pairs in head
            # 2t-1 at jcs 0/2/4/6, swaps (DMA) interleaved at odd jcs.
            for t in (1, 2, 3):
                hk = 2 * t - 1
                add_filler(hk, 0, (lambda t=t: qk_piece("q", qt_lo, t, 0, 0)))
                add_filler(hk, 0, (lambda t=t: qk_piece("q", qt_lo, t, 0, 1)))
                add_filler(hk, 2, (lambda t=t: qk_piece("q", qt_lo, t, 1, 0)))
                add_filler(hk, 2, (lambda t=t: qk_piece("q", qt_lo, t, 1, 1)))
                add_dfiller(hk, 3, (lambda t=t: swap_half(qt_lo, qt_hi, t, 0)))
                add_dfiller(hk, 3, (lambda t=t: swap_half(qt_lo, qt_hi, t, 1)))
                add_filler(hk, 4, (lambda t=t: qk_piece("k", kt_lo, t, 0, 0)))
                add_filler(hk, 4, (lambda t=t: qk_piece("k", kt_lo, t, 0, 1)))
                add_dfiller(hk, 5, (lambda t=t: swap_half(kt_lo, kt_hi, t, 0)))
                add_filler(hk, 6, (lambda t=t: qk_piece("k", kt_lo, t, 1, 0)))
                add_filler(hk, 6, (lambda t=t: qk_piece("k", kt_lo, t, 1, 1)))
                add_dfiller(hk, 7, (lambda t=t: swap_half(kt_lo, kt_hi, t, 1)))

            # out-projection kt0+kt1 partials: 4 pairs in head 6 (ot0/ot1
            # are final after head 3's normalization).
            for i in range(4):
                ch, hf = divmod(i, 2)
                add_filler(6, 2 * i + 1, (lambda ch=ch, hf=hf:
                                          op01_piece(ch, hf)))
                ch2, hf2 = divmod(i + 4, 2)
                add_filler(6, 2 * i + 1, (lambda ch=ch2, hf=hf2:
                                          op01_piece(ch, hf)))
            # warm the GpSimd broadcast ucode shortly before the tail uses
            # it (the library gets evicted; a cold call stalls ~3.4us).
            add_dfiller(7, 10, lambda: nc.gpsimd.partition_broadcast(
                pre_bc[:, :], pre_bc[0:1, :], channels=64))

            # ---- attention, head by head ----
            pv_last = None
            TRAIL = 2
            for h in range(H):
                t, p = h // 2, h % 2
                lo_sl = slice(64 * p, 64 * p + 64)
                hi_sl = slice(64 * (1 - p), 64 * (1 - p) + 64)
                pv = psV.tile([128, 4 * 512], f32, name="pvh", tag="pv")
                ets = {}

                def pv_mms(jc):
                    for it in range(NST):
                        nc.tensor.matmul(
                            pv[0:DH + 1, it * 512:(it + 1) * 512],
                            lhsT=v_aug[jc][:, h * (DH + 1):(h + 1) * (DH + 1)],
                            rhs=ets[jc][:, it * 512:(it + 1) * 512],
                            start=(jc == 0),
                            stop=(jc == NSC - 1),
                        )

                for jc in range(NSC):
                    et = epool.tile([128, S], bf16, name="et", tag="et")
                    ets[jc] = et
                    for half in range(2):
                        pa = psA.tile([128, 1024], f32, name="pa", tag="pa")
                        i0, i1 = 2 * half, 2 * half + 1
                        nc.tensor.matmul(
                            pa[:, 0:512],
                            lhsT=kt_lo[t][lo_sl, jc * 128:(jc + 1) * 128],
                            rhs=qt_lo[t][lo_sl, i0 * 512:(i0 + 1) * 512],
                        )
                        nc.tensor.matmul(
                            pa[:, 512:1024],
                            lhsT=kt_hi[t][hi_sl, jc * 128:(jc + 1) * 128],
                            rhs=qt_hi[t][hi_sl, i1 * 512:(i1 + 1) * 512],
                        )
                        nc.scalar.activation(
                            out=et[:, half * 1024:(half + 1) * 1024],
                            in_=pa[:, :],
                            func=Exp,
                            scale=SCALE,
                        )
                    fl = fillers[h].get(jc, ())
                    if fl:
                        fl[0]()
                    if jc >= TRAIL:
                        pv_mms(jc - TRAIL)
                    for fn in fl[1:]:
                        fn()
                    for fn in dfillers[h].get(jc, ()):
                        fn()
                for jc in range(NSC - TRAIL, NSC):
                    pv_mms(jc)

                if h < H - 1:
                    # decouple normalization: copy O_un+den out of PSUM,
                    # reciprocal via (128,16) reshape, DRAM partition
                    # broadcast, multiply. All hidden under head h+1.
                    oun = spool.tile([DH + 1, S], f32, name="oun", tag="oun")
                    nc.vector.tensor_copy(oun[:, :], pv[0:DH + 1, :])
                    den128 = spool.tile([128, 16], f32, name="den128",
                                        tag="d128")
                    nc.sync.dma_start(out=den128[:, :], in_=oun[DH:DH + 1, :])
                    nc.vector.reciprocal(out=den128[:, :], in_=den128[:, :])
                    nc.sync.dma_start(out=den_dram[h, :], in_=den128[:, :])
                    bc = spool.tile([64, S], f32, name="bc", tag="bc")
                    dd = den_dram[h:h + 1, :]
                    bcast_src = bass.AP(
                        tensor=dd.tensor,
                        offset=dd.offset,
                        ap=[[0, 64]] + [list(x) for x in dd.ap[1:]],
                    )
                    nc.sync.dma_start(out=bc[:, :], in_=bcast_src)
                    nc.vector.tensor_mul(
                        ot[t][64 * p:64 * p + 64, :], oun[0:DH, :], bc[:, :])
                else:
                    pv_last = pv

            # ---- head 7 normalization (critical tail) ----
            # ACT copies the PSUM denominator row out (ACT is idle now),
            # DVE fast-reciprocal on the row, GpSimd partition-broadcast
            # (measured 3.3us — the DRAM round trip stalls ~7us), then
            # block-wise multiplies so the kt matmuls can chase them.
            row7 = spool.tile([DH + 1, S], f32, name="oun", tag="oun")
            nc.scalar.copy(row7[0:1, :], pv_last[DH:DH + 1, :])
            rec7 = spool.tile([DH + 1, S], f32, name="oun", tag="oun")
            nc.vector.reciprocal_approx_fast(out=rec7[0:1, :],
                                             in_=row7[0:1, :])
            bc7 = spool.tile([64, S], f32, name="bc", tag="bc")
            nc.gpsimd.partition_broadcast(bc7[:, :], rec7[0:1, :],
                                          channels=64)
            # keep the PE clock hot through the normalization bubble
            for k in range(13):
                ja = psA.tile([128, 1024], f32, name="pa", tag="pa")
                for half in range(2):
                    nc.tensor.matmul(
                        ja[:, half * 512:(half + 1) * 512],
                        lhsT=junk_sb[:, 0:128],
                        rhs=junk_sb[:, :],
                    )
            for st in range(NST):
                sl = slice(st * 512, (st + 1) * 512)
                nc.vector.tensor_mul(
                    ot[3][64:128, sl], pv_last[0:DH, sl], bc7[:, sl])

            # ---- tail: kt2+kt3 matmuls + (po + bias) + stage combine ----
            # hf-major so half-0 chunks start right after the first two
            # normalize blocks; combines split DVE / ACT+GpSimd to overlap.
            for i in range(8):
                hf, ch = divmod(i, 4)
                po = psA.tile([128, 1024], f32, name="pa", tag="pa")
                for st2 in range(2):
                    st = hf * 2 + st2
                    for kt in (2, 3):
                        nc.tensor.matmul(
                            po[:, st2 * 512:(st2 + 1) * 512],
                            lhsT=wo_s[kt][:, ch * 128:(ch + 1) * 128],
                            rhs=ot[kt][:, st * 512:(st + 1) * 512],
                            start=(kt == 2),
                            stop=(kt == 3),
                        )
                ostage = opool.tile([128, 1024], bf16, name="ost", tag="ost")
                nc.vector.scalar_tensor_tensor(
                    out=ostage[:, :], in0=po[:, :],
                    scalar=bo_s[ch][:, :],
                    in1=stg[ch][hf][:, :], op0=Add, op1=Add)
                nc.sync.dma_start(
                    out=out[ch * 128:(ch + 1) * 128,
                            hf * 1024:(hf + 1) * 1024],
                    in_=ostage[:, :],
                )

    nc.finalize()
    return nc


_NC_CACHE = None


def _get_nc():
    global _NC_CACHE
    if _NC_CACHE is None:
        _NC_CACHE = _build_kernel()
    return _NC_CACHE


def kernel(x, W_qkv, W_out, b_out):
    from concourse.bass_utils import run_bass_kernel_spmd

    bf16 = ml_dtypes.bfloat16

    # head-interleave and transpose the qkv weight: row 192h+{0,64,128}+c of
    # W_qkv is q/k/v row (h, c); regroup to e' = 64h+c and transpose to [d, e']
    w3 = W_qkv.reshape(H, 3, DH, D)
    wq_h = np.ascontiguousarray(w3[:, 0].reshape(INNER, D).T).astype(bf16)
    wk_h = np.ascontiguousarray(w3[:, 1].reshape(INNER, D).T).astype(bf16)
    wv_h = np.ascontiguousarray(w3[:, 2].reshape(INNER, D).T).astype(bf16)
    wo_h = np.ascontiguousarray(W_out.T).astype(bf16)  # [hc, d]
    bo_h = np.ascontiguousarray(b_out.reshape(NDT, 128, 1)).astype(np.float32)

    in_maps = []
    for b in range(N_CORES):
        xT_b = np.ascontiguousarray(x[b].T).astype(bf16)  # [d, s]
        in_maps.append({
            "xT": xT_b, "wq": wq_h, "wk": wk_h, "wv": wv_h,
            "wo": wo_h, "bo": bo_h,
        })

    nc = _get_nc()
    res = run_bass_kernel_spmd(nc, in_maps, list(range(N_CORES)))
    outs = [res.results[b]["out"].T for b in range(N_CORES)]  # [s, d] each
    return np.ascontiguousarray(np.stack(outs, axis=0)).astype(np.float32)


# revision 38
# speedup vs baseline: 1.0384x; 1.0138x over previous
"""Multi-head attention (B=8, S=2048, D=512, H=8, DH=64) on 8 TRN2 NeuronCores.

Data-parallel over batch; per core everything is transposed (feature on
partitions) so softmax reductions ride the TensorE contraction axis.

v2 schedule (trace-driven rework of the staged baseline):
  - Steady state is ACT-bound at ~1005ns per [128,1024] exp; PE runs
    ~830-940ns/half-jc, so filler projection work must be fine-grained.
  - PV trails the exp stream by TWO j-chunks (was 1), so PV matmuls never
    wait on the just-issued exp (-120ns/jc of PE stall).
  - Lead-in only computes what head 0 jc0 needs: Q pair-0 (full), K pair-0
    column block 0, V chunks 0-3; wq/wk pair-0 column slices are loaded as
    separate tiles so the first matmuls don't wait on the full weight DMA.
    K blocks 1-3, V 4-15 become early head-0 fillers with per-jc deadlines.
  - Output projection is split by contraction block: kt0+kt1 accumulate
    into PSUM during heads 5-6 and stage to SBUF (bf16), kt2 adds in during
    head 7, so the tail only runs kt3 matmuls + one fused
    (po + bias) + stage combine per output chunk (DVE/GpSimd split).
  - Head-7 normalization avoids the slow [1,2048] single-partition ops:
    DMA the PSUM denominator row into a [128,16] tile, reciprocal there,
    DRAM partition-broadcast back, multiply out of PSUM.
"""

import numpy as np
import ml_dtypes

B, S, D = 8, 2048, 512
H, DH = 8, 64
INNER = H * DH
SCALE = DH ** -0.5

N_CORES = 8
NDT = D // 128   # 4 contraction tiles
NSC = S // 128   # 16 s-chunks (j-chunks)
NST = S // 512   # 4 s-tiles


def _build_kernel():
    import concourse.bass as bass
    import concourse.mybir as mybir
    import concourse.tile as tile
    from concourse import bacc

    bf16 = mybir.dt.bfloat16
    f32 = mybir.dt.float32
    Exp = mybir.ActivationFunctionType.Exp
    Add = mybir.AluOpType.add

    nc = bacc.Bacc()

    xT = nc.declare_dram_parameter("xT", [D, S], bf16, isOutput=False)
    wq = nc.declare_dram_parameter("wq", [D, INNER], bf16, isOutput=False)
    wk = nc.declare_dram_parameter("wk", [D, INNER], bf16, isOutput=False)
    wv = nc.declare_dram_parameter("wv", [D, INNER], bf16, isOutput=False)
    wo = nc.declare_dram_parameter("wo", [INNER, D], bf16, isOutput=False)
    bo = nc.declare_dram_parameter("bo", [NDT, 128, 1], f32, isOutput=False)
    out = nc.declare_dram_parameter("out", [D, S], bf16, isOutput=True)
    den_dram = nc.dram_tensor("den_scratch", [H, S], f32)

    with tile.TileContext(nc) as tc:
        with (
            tc.tile_pool(name="weights", bufs=1) as wpool,
            tc.tile_pool(name="acts", bufs=1) as apool,
            tc.tile_pool(name="et", bufs=4) as epool,
            tc.tile_pool(name="small", bufs=2) as spool,
            tc.tile_pool(name="ostage", bufs=2) as opool,
            tc.tile_pool(name="psA", bufs=2, space="PSUM") as psA,
            tc.tile_pool(name="psV", bufs=1, space="PSUM") as psV,
        ):
            # ---- SBUF tiles ----
            # wq/wk pair-0 column slice gets its own tile so lead-in matmuls
            # depend only on the small early DMA, not the full weight load.
            wqA = [wpool.tile([128, 128], bf16, name=f"wqA{d}", tag=f"wqA{d}")
                   for d in range(NDT)]
            wkA = [wpool.tile([128, 128], bf16, name=f"wkA{d}", tag=f"wkA{d}")
                   for d in range(NDT)]
            wqB = [wpool.tile([128, INNER - 128], bf16, name=f"wqB{d}",
                              tag=f"wqB{d}") for d in range(NDT)]
            wkB = [wpool.tile([128, INNER - 128], bf16, name=f"wkB{d}",
                              tag=f"wkB{d}") for d in range(NDT)]
            xT_s = [[wpool.tile([128, S // 2], bf16, name=f"xT{d}_{hf}",
                                tag=f"xT{d}_{hf}") for hf in range(2)]
                    for d in range(NDT)]
            wv_s = [wpool.tile([128, INNER], bf16, name=f"wv{d}", tag=f"wv{d}")
                    for d in range(NDT)]
            wo_s = [wpool.tile([128, D], bf16, name=f"wo{d}", tag=f"wo{d}")
                    for d in range(NDT)]
            bo_s = [wpool.tile([128, 1], f32, name=f"bo{d}", tag=f"bo{d}")
                    for d in range(NDT)]

            def wsel(w_pair, t):
                """(tile list, col offset) for head-pair t's projection."""
                if t == 0:
                    return (wqA if w_pair == "q" else wkA), 0
                return (wqB if w_pair == "q" else wkB), (t - 1) * 128

            # ---- prioritized input DMA ----
            # Descriptor generation costs ~600ns of the ISSUING engine's
            # sequencer per dma_start, so keep the ACT (scalar) sequencer
            # completely free of DMA work — a clogged ACT queue delays the
            # first exp by microseconds. xT halves gate Q0: sync + vector.
            for d in range(NDT):
                sl = slice(d * 128, (d + 1) * 128)
                nc.sync.dma_start(out=xT_s[d][0][:], in_=xT[sl, 0:S // 2])
                nc.gpsimd.dma_start(out=wqA[d][:], in_=wq[sl, 0:128])
            for d in range(NDT):
                sl = slice(d * 128, (d + 1) * 128)
                nc.sync.dma_start(out=xT_s[d][1][:], in_=xT[sl, S // 2:])
                nc.gpsimd.dma_start(out=wkA[d][:], in_=wk[sl, 0:128])
            for d in range(NDT):
                sl = slice(d * 128, (d + 1) * 128)
                nc.sync.dma_start(out=wv_s[d][:], in_=wv[sl, :])
            for d in range(NDT):
                sl = slice(d * 128, (d + 1) * 128)
                nc.gpsimd.dma_start(out=wo_s[d][:], in_=wo[sl, :])
                nc.gpsimd.dma_start(out=bo_s[d][:], in_=bo[d, :, :])
                nc.gpsimd.dma_start(out=wqB[d][:], in_=wq[sl, 128:])
                nc.gpsimd.dma_start(out=wkB[d][:], in_=wk[sl, 128:])

            # ---- QKV target tiles ----
            qt_lo = [apool.tile([128, S], bf16, name=f"qlo{t}", tag=f"qlo{t}")
                     for t in range(NDT)]
            kt_lo = [apool.tile([128, S], bf16, name=f"klo{t}", tag=f"klo{t}")
                     for t in range(NDT)]
            qt_hi = [apool.tile([128, S], bf16, name=f"qhi{t}", tag=f"qhi{t}")
                     for t in range(NDT)]
            kt_hi = [apool.tile([128, S], bf16, name=f"khi{t}", tag=f"khi{t}")
                     for t in range(NDT)]
            v_aug = [apool.tile([128, H * (DH + 1)], bf16, name=f"va{m}",
                                tag=f"va{m}") for m in range(NSC)]
            ot = [apool.tile([128, S], bf16, name=f"ot{t}", tag=f"ot{t}")
                  for t in range(NDT)]
            # out-projection kt{0,1}+kt2 partial sums, bf16 staging
            stg = [[wpool.tile([128, 1024], bf16, name=f"stg{ch}_{hf}",
                               tag=f"stg{ch}_{hf}") for hf in range(2)]
                   for ch in range(NDT)]

            # PE warm-up + GpSimd ucode preload (lazy LOAD_LIB is ~7us)
            junk_sb = wpool.tile([128, 512], bf16, name="junk", tag="junk")
            nc.vector.memset(junk_sb[:, :], 0.0)
            pre_bc = wpool.tile([64, 16], f32, name="prebc", tag="prebc")
            nc.vector.memset(pre_bc[:, :], 1.0)
            nc.gpsimd.partition_broadcast(pre_bc[:, :], pre_bc[0:1, :],
                                          channels=64)
            junk_ps = psV.tile([128, 4 * 512], f32, name="junkps", tag="pv")
            _junk_k = [0]

            def junk_fill(n, wide=True):
                """n matmuls into the junk PSUM tile to keep the HAM clock
                ramp alive across lead-in DMA waits."""
                for _ in range(n):
                    k = _junk_k[0] = _junk_k[0] + 1
                    nc.tensor.matmul(
                        junk_ps[:, (k % 4) * 512:(k % 4 + 1) * 512],
                        lhsT=junk_sb[:, 0:128],
                        rhs=junk_sb[:, 0:512] if wide else junk_sb[:, 0:128],
                    )

            junk_fill(16)

            # ---- projection piece helpers ----
            def qk_half(w_pair, dst, t, half):
                """One s-half (1024 cols) of a Q/K projection: 8 matmuls."""
                w_s, co = wsel(w_pair, t)
                pa = psA.tile([128, 1024], f32, name="pa", tag="pa")
                for nn in range(2):
                    s0 = nn * 512
                    for d in range(NDT):
                        nc.tensor.matmul(
                            pa[:, nn * 512:(nn + 1) * 512],
                            lhsT=w_s[d][:, co:co + 128],
                            rhs=xT_s[d][half][:, s0:s0 + 512],
                            start=(d == 0),
                            stop=(d == NDT - 1),
                        )
                nc.vector.tensor_copy(
                    dst[t][:, half * 1024:(half + 1) * 1024], pa[:, :])

            def qk_piece(w_pair, dst, t, half, nn):
                """4-matmul piece: one 512-col block of a Q/K projection."""
                w_s, co = wsel(w_pair, t)
                pa = psA.tile([128, 1024], f32, name="pa", tag="pa")
                s0 = nn * 512
                for d in range(NDT):
                    nc.tensor.matmul(
                        pa[:, 0:512],
                        lhsT=w_s[d][:, co:co + 128],
                        rhs=xT_s[d][half][:, s0:s0 + 512],
                        start=(d == 0),
                        stop=(d == NDT - 1),
                    )
                off = half * 1024 + nn * 512
                nc.vector.tensor_copy(dst[t][:, off:off + 512], pa[:, 0:512])

            def swap_half(src_lo, src_hi, t, sh, eng=None):
                """DMA-swap one 1024-wide s-half of the lo replica into hi."""
                eng = eng or nc.gpsimd
                s0, s1 = sh * 1024, (sh + 1) * 1024
                eng.dma_start(out=src_hi[t][64:128, s0:s1],
                              in_=src_lo[t][0:64, s0:s1])
                eng.dma_start(out=src_hi[t][0:64, s0:s1],
                              in_=src_lo[t][64:128, s0:s1])

            def swap_block(src_lo, src_hi, t, half, nn, eng=None):
                """DMA-swap one 512-col block (for just-in-time K pair 0)."""
                eng = eng or nc.gpsimd
                s0 = half * 1024 + nn * 512
                s1 = s0 + 512
                eng.dma_start(out=src_hi[t][64:128, s0:s1],
                              in_=src_lo[t][0:64, s0:s1])
                eng.dma_start(out=src_hi[t][0:64, s0:s1],
                              in_=src_lo[t][64:128, s0:s1])

            def v_fill(dst_ap):
                return dst_ap.rearrange("p (h t) -> p h t", t=DH + 1)

            def v_piece(m):
                """4-matmul filler piece: V projection for j-chunk m."""
                pa = psA.tile([128, 1024], f32, name="pa", tag="pa")
                mh, mo = divmod(m, 8)
                for d in range(NDT):
                    nc.tensor.matmul(
                        pa[:, 0:512],
                        lhsT=xT_s[d][mh][:, mo * 128:(mo + 1) * 128],
                        rhs=wv_s[d][:, :],
                        start=(d == 0),
                        stop=(d == NDT - 1),
                    )
                va = v_fill(v_aug[m])
                nc.vector.tensor_copy(
                    va[:, :, 0:DH],
                    pa[:, 0:512].rearrange("p (h t) -> p h t", t=DH),
                )
                nc.vector.memset(va[:, :, DH:DH + 1], 1.0)

            def op01_piece(ch, hf):
                """Out-projection kt0+kt1 into PSUM, staged to SBUF bf16."""
                pa = psA.tile([128, 1024], f32, name="pa", tag="pa")
                for st2 in range(2):
                    st = hf * 2 + st2
                    for kt in range(2):
                        nc.tensor.matmul(
                            pa[:, st2 * 512:(st2 + 1) * 512],
                            lhsT=wo_s[kt][:, ch * 128:(ch + 1) * 128],
                            rhs=ot[kt][:, st * 512:(st + 1) * 512],
                            start=(kt == 0),
                            stop=(kt == 1),
                        )
                nc.vector.tensor_copy(stg[ch][hf][:, :], pa[:, :])

            # ---- lead-in compute ----
            # Only what scores(jc0, half0) needs: Q pair-0 columns 0:1024
            # (direct block + swapped block) and K block 0. The other Q
            # half is injected mid-jc0 under the first exp; everything
            # else is loop filler. Junk matmuls keep the clock ramp alive
            # across DMA waits.
            qk_piece("q", qt_lo, 0, 0, 0)
            junk_fill(2)
            qk_piece("q", qt_lo, 0, 0, 1)
            swap_block(qt_lo, qt_hi, 0, 0, 1, eng=nc.sync)
            junk_fill(2)
            qk_piece("k", kt_lo, 0, 0, 0)
            swap_block(kt_lo, kt_hi, 0, 0, 0, eng=nc.sync)

            # ---- filler schedule ----
            # Fillers that allocate a psA tile MUST come in pairs per jc
            # (one before PV, one after): an odd allocation count flips the
            # psA rotation parity and puts the next jc's scores in WAR with
            # the LATE exp (measured ~727ns ACT stall per lone piece vs
            # ~370ns for a paired one). DMA-only fillers (swaps) are free.
            fillers = {h: {} for h in range(H)}   # h -> jc -> [psA pieces]
            dfillers = {h: {} for h in range(H)}  # h -> jc -> [DMA thunks]

            def add_filler(h, jc, fn):
                fillers[h].setdefault(jc, []).append(fn)

            def add_dfiller(h, jc, fn):
                dfillers[h].setdefault(jc, []).append(fn)

            # head 0: V chunk m as a singleton at jc m+1 (consumed by PV(m)
            # at loop-jc m+2); V15 doubles up at jc15. K pair-0 blocks ride
            # along at jcs 2/6/10 (deadlines 4/8/12).
            for m in range(15):
                add_filler(0, m + 1, (lambda m=m: v_piece(m)))
            add_filler(0, 15, lambda: v_piece(15))
            add_filler(0, 6, lambda: qk_piece("k", kt_lo, 0, 1, 0))
            add_dfiller(0, 6, lambda: swap_block(kt_lo, kt_hi, 0, 1, 0))
            add_filler(0, 10, lambda: qk_piece("k", kt_lo, 0, 1, 1))
            add_dfiller(0, 10, lambda: swap_block(kt_lo, kt_hi, 0, 1, 1))

            # Q/K projection for pair t, all singletons. Pair 1 fits in
            # head 1; pairs 2/3 split Q into head 2t-2 and K into 2t-1.
            def qk_sched(t, slots):
                hq, hk = slots
                add_filler(hq[0][0], hq[0][1],
                           (lambda t=t: qk_piece("q", qt_lo, t, 0, 0)))
                add_filler(hq[1][0], hq[1][1],
                           (lambda t=t: qk_piece("q", qt_lo, t, 0, 1)))
                add_filler(hq[2][0], hq[2][1],
                           (lambda t=t: qk_piece("q", qt_lo, t, 1, 0)))
                add_filler(hq[3][0], hq[3][1],
                           (lambda t=t: qk_piece("q", qt_lo, t, 1, 1)))
                add_dfiller(hq[3][0], hq[3][1] + 1,
                            (lambda t=t: swap_half(qt_lo, qt_hi, t, 0)))
                add_dfiller(hq[3][0], hq[3][1] + 1,
                            (lambda t=t: swap_half(qt_lo, qt_hi, t, 1)))
                add_filler(hk[0][0], hk[0][1],
                           (lambda t=t: qk_piece("k", kt_lo, t, 0, 0)))
                add_filler(hk[1][0], hk[1][1],
                           (lambda t=t: qk_piece("k", kt_lo, t, 0, 1)))
                add_dfiller(hk[1][0], hk[1][1] + 1,
                            (lambda t=t: swap_half(kt_lo, kt_hi, t, 0)))
                add_filler(hk[2][0], hk[2][1],
                           (lambda t=t: qk_piece("k", kt_lo, t, 1, 0)))
                add_filler(hk[3][0], hk[3][1],
                           (lambda t=t: qk_piece("k", kt_lo, t, 1, 1)))
                add_dfiller(hk[3][0], hk[3][1] + 1,
                            (lambda t=t: swap_half(kt_lo, kt_hi, t, 1)))

            qk_sched(1, ([(1, 0), (1, 2), (1, 4), (1, 6)],
                         [(1, 8), (1, 10), (1, 12), (1, 14)]))
            qk_sched(2, ([(2, 2), (2, 5), (2, 8), (2, 11)],
                         [(3, 1), (3, 4), (3, 7), (3, 10)]))
            qk_sched(3, ([(4, 2), (4, 5), (4, 8), (4, 11)],
                         [(5, 1), (5, 4), (5, 7), (5, 10)]))

            # warm the GpSimd broadcast ucode shortly before the tail uses
            # it (the library gets evicted; a cold call stalls ~3.4us).
            add_dfiller(7, 10, lambda: nc.gpsimd.partition_broadcast(
                pre_bc[:, :], pre_bc[0:1, :], channels=64))

            # ---- attention, head by head ----
            pv_last = None
            TRAIL = 2
            for h in range(H):
                t, p = h // 2, h % 2
                lo_sl = slice(64 * p, 64 * p + 64)
                hi_sl = slice(64 * (1 - p), 64 * (1 - p) + 64)
                pv = psV.tile([128, 4 * 512], f32, name="pvh", tag="pv")
                ets = {}

                def pv_mms(jc):
                    for it in range(NST):
                        nc.tensor.matmul(
                            pv[0:DH + 1, it * 512:(it + 1) * 512],
                            lhsT=v_aug[jc][:, h * (DH + 1):(h + 1) * (DH + 1)],
                            rhs=ets[jc][:, it * 512:(it + 1) * 512],
                            start=(jc == 0),
                            stop=(jc == NSC - 1),
                        )

                for jc in range(NSC):
                    et = epool.tile([128, S], bf16, name="et", tag="et")
                    ets[jc] = et
                    for half in range(2):
                        pa = psA.tile([128, 1024], f32, name="pa", tag="pa")
                        i0, i1 = 2 * half, 2 * half + 1
                        nc.tensor.matmul(
                            pa[:, 0:512],
                            lhsT=kt_lo[t][lo_sl, jc * 128:(jc + 1) * 128],
                            rhs=qt_lo[t][lo_sl, i0 * 512:(i0 + 1) * 512],
                        )
                        nc.tensor.matmul(
                            pa[:, 512:1024],
                            lhsT=kt_hi[t][hi_sl, jc * 128:(jc + 1) * 128],
                            rhs=qt_hi[t][hi_sl, i1 * 512:(i1 + 1) * 512],
                        )
                        nc.scalar.activation(
                            out=et[:, half * 1024:(half + 1) * 1024],
                            in_=pa[:, :],
                            func=Exp,
                            scale=SCALE,
                        )
                        if h == 0 and jc == 0 and half == 0:
                            # finish Q pair-0 under the very first exp
                            qk_piece("q", qt_lo, 0, 1, 0)
                            qk_piece("q", qt_lo, 0, 1, 1)
                            swap_block(qt_lo, qt_hi, 0, 1, 1, eng=nc.sync)
                            qk_piece("k", kt_lo, 0, 0, 1)
                            swap_block(kt_lo, kt_hi, 0, 0, 1, eng=nc.sync)
                    fl = fillers[h].get(jc, ())
                    if fl:
                        fl[0]()
                    if jc >= TRAIL:
                        pv_mms(jc - TRAIL)
                    for fn in fl[1:]:
                        fn()
                    for fn in dfillers[h].get(jc, ()):
                        fn()
                for jc in range(NSC - TRAIL, NSC):
                    pv_mms(jc)

                if h < H - 1:
                    # decouple normalization: copy O_un+den out of PSUM,
                    # reciprocal via (128,16) reshape, DRAM partition
                    # broadcast, multiply. All hidden under head h+1.
                    oun = spool.tile([DH + 1, S], f32, name="oun", tag="oun")
                    nc.vector.tensor_copy(oun[:, :], pv[0:DH + 1, :])
                    den128 = spool.tile([128, 16], f32, name="den128",
                                        tag="d128")
                    nc.sync.dma_start(out=den128[:, :], in_=oun[DH:DH + 1, :])
                    nc.vector.reciprocal(out=den128[:, :], in_=den128[:, :])
                    nc.sync.dma_start(out=den_dram[h, :], in_=den128[:, :])
                    bc = spool.tile([64, S], f32, name="bc", tag="bc")
                    dd = den_dram[h:h + 1, :]
                    bcast_src = bass.AP(
                        tensor=dd.tensor,
                        offset=dd.offset,
                        ap=[[0, 64]] + [list(x) for x in dd.ap[1:]],
                    )
                    nc.sync.dma_start(out=bc[:, :], in_=bcast_src)
                    nc.vector.tensor_mul(
                        ot[t][64 * p:64 * p + 64, :], oun[0:DH, :], bc[:, :])
                else:
                    pv_last = pv

            # ---- head 7 normalization (critical tail) ----
            # DVE fast-reciprocal straight out of the PSUM denominator row,
            # GpSimd partition-broadcast, block-wise multiplies. The
            # kt0+kt1 out-projection partials fill the PE during this
            # bubble, with their PSUM->stage copies on the otherwise-idle
            # ACT engine.
            row7 = spool.tile([DH + 1, S], f32, name="oun", tag="oun")
            nc.scalar.copy(row7[0:1, :], pv_last[DH:DH + 1, :])
            rec7 = spool.tile([DH + 1, S], f32, name="oun", tag="oun")
            nc.vector.reciprocal_approx_fast(out=rec7[0:1, :],
                                             in_=row7[0:1, :])
            bc7 = spool.tile([64, S], f32, name="bc", tag="bc")
            nc.gpsimd.partition_broadcast(bc7[:, :], rec7[0:1, :],
                                          channels=64)
            for i in range(8):
                ch, hf = divmod(i, 2)
                pa = psA.tile([128, 1024], f32, name="pa", tag="pa")
                for st2 in range(2):
                    st = hf * 2 + st2
                    for kt in range(2):
                        nc.tensor.matmul(
                            pa[:, st2 * 512:(st2 + 1) * 512],
                            lhsT=wo_s[kt][:, ch * 128:(ch + 1) * 128],
                            rhs=ot[kt][:, st * 512:(st + 1) * 512],
                            start=(kt == 0),
                            stop=(kt == 1),
                        )
                nc.scalar.copy(stg[ch][hf][:, :], pa[:, :])
            for st in range(NST):
                sl = slice(st * 512, (st + 1) * 512)
                nc.vector.tensor_mul(
                    ot[3][64:128, sl], pv_last[0:DH, sl], bc7[:, sl])

            # ---- tail: kt2+kt3 matmuls + (po + bias) + stage combine ----
            # hf-major so half-0 chunks start right after the first two
            # normalize blocks.
            for i in range(8):
                hf, ch = divmod(i, 4)
                po = psA.tile([128, 1024], f32, name="pa", tag="pa")
                for st2 in range(2):
                    st = hf * 2 + st2
                    for kt in (2, 3):
                        nc.tensor.matmul(
                            po[:, st2 * 512:(st2 + 1) * 512],
                            lhsT=wo_s[kt][:, ch * 128:(ch + 1) * 128],
                            rhs=ot[kt][:, st * 512:(st + 1) * 512],
                            start=(kt == 2),
                            stop=(kt == 3),
                        )
                ostage = opool.tile([128, 1024], bf16, name="ost", tag="ost")
                nc.vector.scalar_tensor_tensor(
                    out=ostage[:, :], in0=po[:, :],
                    scalar=bo_s[ch][:, :],
                    in1=stg[ch][hf][:, :], op0=Add, op1=Add)
                (nc.sync, nc.gpsimd)[i % 2].dma_start(
                    out=out[ch * 128:(ch + 1) * 128,
                            hf * 1024:(hf + 1) * 1024],
                    in_=ostage[:, :],
                )

    nc.finalize()
    return nc


_NC_CACHE = None


def _get_nc():
    global _NC_CACHE
    if _NC_CACHE is None:
        _NC_CACHE = _build_kernel()
    return _NC_CACHE


def kernel(x, W_qkv, W_out, b_out):
    from concourse.bass_utils import run_bass_kernel_spmd

    bf16 = ml_dtypes.bfloat16

    # head-interleave and transpose the qkv weight: row 192h+{0,64,128}+c of
    # W_qkv is q/k/v row (h, c); regroup to e' = 64h+c and transpose to [d, e']
    w3 = W_qkv.reshape(H, 3, DH, D)
    wq_h = np.ascontiguousarray(w3[:, 0].reshape(INNER, D).T).astype(bf16)
    wk_h = np.ascontiguousarray(w3[:, 1].reshape(INNER, D).T).astype(bf16)
    wv_h = np.ascontiguousarray(w3[:, 2].reshape(INNER, D).T).astype(bf16)
    wo_h = np.ascontiguousarray(W_out.T).astype(bf16)  # [hc, d]
    bo_h = np.ascontiguousarray(b_out.reshape(NDT, 128, 1)).astype(np.float32)

    in_maps = []
    for b in range(N_CORES):
        xT_b = np.ascontiguousarray(x[b].T).astype(bf16)  # [d, s]
        in_maps.append({
            "xT": xT_b, "wq": wq_h, "wk": wk_h, "wv": wv_h,
            "wo": wo_h, "bo": bo_h,
        })

    nc = _get_nc()
    res = run_bass_kernel_spmd(nc, in_maps, list(range(N_CORES)))
    outs = [res.results[b]["out"].T for b in range(N_CORES)]  # [s, d] each
    return np.ascontiguousarray(np.stack(outs, axis=0)).astype(np.float32)
